# revision 8
# baseline (speedup 1.0000x reference)
"""Trainium2 Bass kernel for the Compressor module (sparse-attention KV
compression): fused kv/score projections -> overlapped softmax pooling ->
RMSNorm -> RoPE.

Sharding: data-parallel over (batch x seq-half) across 8 cores. Each core
processes 2048 tokens of one batch with a 4-token halo at the front, so no
collectives are needed. Weights are replicated.

Layout: matmuls compute out.T = W @ x.T ([channel, token]). x.T is resident
in SBUF (32 bf16 k-rows of [128, 2052]); weight tiles stream through a
3-deep pool, each used for 4 moving matmuls (one per 512-token chunk), so
LdWeights is amortized 4x. One output-channel tile (4 psum banks) is in
flight at a time, ping-ponging with the previous tile's drain. The
"overlap" halves come from a 4-token shifted matmul window (lo channels
cols [0,2048), hi cols [4,2052)), which makes every softmax-pool group of
4 column-aligned.

Precision: kv matmuls all bf16 (~0.3% err). Score matmuls are split-K:
first 2560 dims bf16 with weights pre-scaled by 2048, last 1536 dims in
fp8-e4m3 DoubleRow perf mode (2 K-rows/cycle) with x*8 / w*256 scaling --
the psum ends up at 2048x scale, undone for free by the Exp activation's
scale argument. Pool/epilogue intermediates in bf16 where harmless.
Predicted rel err ~1.5e-2 vs the 2e-2 budget (validated numerically
against the real inputs; the numeric sim matches hardware to 6 digits).

Scheduling: PE-array transposes are deferred until after the NEXT oc's
matmuls are emitted so the tensor queue never waits on the vector drain
chain (which would drop the PE to half-clock pstate). The last oc runs
chunk-major with drains one chunk behind for the same reason. Weight DMAs
ride the fast sync/scalar queues ahead of the x stream; a tiny warm-up
matmul gated on x-row 3 delays the tensor start just enough that the x
stream stays ahead of consumption.
"""

import numpy as np
import ml_dtypes

import concourse.bass as bass
import concourse.mybir as mybir
from concourse import bacc
from concourse.tile import TileContext
from concourse.masks import make_identity
from concourse.bass_utils import run_bass_kernel_spmd

B, S, DIM = 4, 4096, 4096
D, RD, RATIO = 512, 64, 4
EPS = 1e-6
NCORES = 8
TOK = 2048          # tokens per core
XW = TOK + 4        # x window width (4-token halo at the front)
KBF = 22            # bf16 k-tiles for score ocs (kv uses all 32)
KP8 = 5             # fp8 DoubleRow k-pair-tiles for score ocs
SCALE = 2048.0      # score psum scale (SX * SW)
SX, SW = 8.0, 256.0
NEG = -1.0e30
FP = mybir.dt.float32
BF = mybir.dt.bfloat16
F8 = mybir.dt.float8e4
DR = mybir.MatmulPerfMode.DoubleRow
AX = mybir.AxisListType
ALU = mybir.AluOpType
ACTF = mybir.ActivationFunctionType

# oc tiles: 0..7 = kv channels (lo 0..3, hi 4..7), 8..15 = gate/score
# channels (lo 8..11, hi 12..15). Family jj uses {jj, 4+jj, 8+jj, 12+jj}.
OC_ORDER = [8, 12, 0, 4,
            9, 13, 1, 5,
            10, 14, 2, 6,
            11, 15, 3, 7]


def _oc_off(oc: int) -> int:
    """Token-window offset: lo channels read cols [0,2048), hi [4,2052)."""
    return 4 if (oc // 4) % 2 == 1 else 0


def _build_program() -> bass.Bass:
    nc = bacc.Bacc("TRN2", target_bir_lowering=False, debug=False)

    xt = nc.dram_tensor("xt", [32, 128, XW], BF, kind="ExternalInput").ap()
    x8 = nc.dram_tensor("x8", [KP8, 128, 2, XW], F8,
                        kind="ExternalInput").ap()
    w = nc.dram_tensor("w", [16, 128, 32, 128], BF,
                       kind="ExternalInput").ap()
    w8 = nc.dram_tensor("w8", [8, 128, KP8, 2, 128], F8,
                        kind="ExternalInput").ap()
    ape = nc.dram_tensor("ape", [8, 128, 4], FP, kind="ExternalInput").ap()
    cosp = nc.dram_tensor("cosp", [512, 32], FP, kind="ExternalInput").ap()
    sinp = nc.dram_tensor("sinp", [512, 32], FP, kind="ExternalInput").ap()
    normb = nc.dram_tensor("normb", [128, 512], BF, kind="ExternalInput").ap()
    scfix = nc.dram_tensor("scfix", [128, 4], FP, kind="ExternalInput").ap()
    out = nc.dram_tensor("out", [512, 512], FP, kind="ExternalOutput").ap()

    with TileContext(nc) as tc:
        with (
            tc.tile_pool(name="const", bufs=1) as constp,
            tc.tile_pool(name="xp", bufs=32) as xp,
            tc.tile_pool(name="x8p", bufs=KP8) as x8p,
            tc.tile_pool(name="wp", bufs=3) as wp,
            tc.tile_pool(name="w8p", bufs=2) as w8p,
            tc.tile_pool(name="ep", bufs=8) as ep,
            tc.tile_pool(name="tp", bufs=2) as tp,
            tc.tile_pool(name="pp", bufs=2) as pp,
            tc.tile_pool(name="sp", bufs=6) as sp,
            tc.tile_pool(name="yp", bufs=4) as yp,
            tc.tile_pool(name="pmm", bufs=8, space="PSUM") as pmm,
        ):
            ident = constp.tile([128, 128], FP)
            make_identity(nc, ident)
            norm_sb = constp.tile([128, 512], BF)
            nc.gpsimd.dma_start(norm_sb, normb)
            fix_sb = constp.tile([128, 4], FP)
            nc.gpsimd.dma_start(fix_sb, scfix)
            ape_sb = constp.tile([128, 8, 4], FP)
            nc.gpsimd.dma_start(ape_sb, ape.rearrange("j p q -> p j q"))
            eps_sb = constp.tile([128, 1], FP)
            nc.gpsimd.memset(eps_sb[:], EPS)
            cos_sb, sin_sb = {}, {}
            for c in range(4):
                t = constp.tile([128, 32], FP, name=f"cos{c}")
                nc.gpsimd.dma_start(t, cosp[128 * c: 128 * c + 128, :])
                cos_sb[c] = t
                t = constp.tile([128, 32], FP, name=f"sin{c}")
                nc.gpsimd.dma_start(t, sinp[128 * c: 128 * c + 128, :])
                sin_sb[c] = t

            w_sb, w8_sb = {}, {}

            def load_w(oc, eng):
                t = wp.tile([128, 32, 128], BF, tag="wt", name="wt")
                if oc < 8:
                    eng.dma_start(t, w[oc])
                else:
                    eng.dma_start(t[:, 0:KBF, :], w[oc, :, 0:KBF, :])
                    t8 = w8p.tile([128, KP8, 2, 128], F8, tag="w8",
                                  name="w8t")
                    eng.dma_start(t8, w8[oc - 8])
                    w8_sb[oc] = t8
                w_sb[oc] = t

            # weight tiles for the first two ocs at the head of the two
            # fast DMA queues, then the x stream (fp8 rows first: small,
            # and needed mid-phase-A)
            load_w(OC_ORDER[0], nc.sync)
            load_w(OC_ORDER[1], nc.scalar)

            xts = [None] * 32
            x8ts = [None] * KP8

            def load_x(k, eng):
                t = xp.tile([128, XW], BF, tag="xt", name="xtile")
                eng.dma_start(t, xt[k])
                xts[k] = t

            for k in range(8):
                load_x(k, nc.sync if k % 2 == 0 else nc.scalar)
            for kk in range(KP8):
                t = x8p.tile([128, 2, XW], F8, tag="x8t", name="x8tile")
                eng = nc.sync if kk % 2 == 0 else nc.scalar
                eng.dma_start(t, x8[kk])
                x8ts[kk] = t
            for k in range(8, 30):
                load_x(k, nc.sync if k % 2 == 0 else nc.scalar)
            load_x(30, nc.gpsimd)
            load_x(31, nc.gpsimd)

            # warm-up: tiny matmul gated on x-row 3 so the tensor engine
            # wakes only once the x stream has a head start
            warm = pmm.tile([128, 512], FP, tag="ps", name="warm")
            nc.tensor.matmul(warm[0:8, 0:8], lhsT=w_sb[8][:, 0, 0:8],
                             rhs=xts[2][:, 0:8], start=True, stop=True,
                             skip_group_check=True)

            ys = {}
            for c in range(4):
                ys[c] = yp.tile([128, 512], BF, tag="y", name="y")

            def emit_mms(oc, pss, ks=None, chunks=(0, 1, 2, 3),
                         with_f8=True):
                """Matmul stream for one oc into per-chunk psums."""
                off = _oc_off(oc)
                is_score = oc >= 8
                nk = KBF if is_score else 32
                for k in (range(nk) if ks is None else ks):
                    lhsT = w_sb[oc][:, k, :]
                    for c in chunks:
                        nc.tensor.matmul(
                            pss[c], lhsT=lhsT,
                            rhs=xts[k][:, off + 512 * c: off + 512 * c + 512],
                            start=(k == 0),
                            stop=(not is_score and k == 31),
                            skip_group_check=True)
                if is_score and with_f8:
                    for kk in range(KP8):
                        lhsT8 = w8_sb[oc][:, kk, :, :]
                        for c in chunks:
                            nc.tensor.matmul(
                                pss[c], lhsT=lhsT8,
                                rhs=x8ts[kk][:, :,
                                             off + 512 * c:
                                             off + 512 * c + 512],
                                start=False, stop=(kk == KP8 - 1),
                                perf_mode=DR, skip_group_check=True)

            def mm_oc(oc):
                pss = [pmm.tile([128, 512], FP, tag="ps", name="ps")
                       for _ in range(4)]
                emit_mms(oc, pss)
                return pss

            def drain_score(jj, hi, pss, st):
                ap_idx = 4 + jj if hi else jj
                for c in range(4):
                    t = tp.tile([128, 512], FP, tag="tt", name="tt")
                    nc.vector.tensor_tensor(
                        t[:].rearrange("p (b s) -> p b s", s=4),
                        pss[c][:].rearrange("p (b s) -> p b s", s=4),
                        ape_sb[:, ap_idx, None, :].to_broadcast((128, 128, 4)),
                        ALU.add)
                    if (not hi) and c == 0:
                        # first block of the shard: -inf for the missing
                        # previous block (no-op on odd cores)
                        nc.vector.tensor_tensor(t[:, 0:4], t[:, 0:4],
                                                fix_sb[:], ALU.add)
                    e = ep.tile([128, 512], BF, tag="et", name="et")
                    nc.scalar.activation(e[:], t[:], ACTF.Exp,
                                         scale=1.0 / SCALE)
                    d = sp.tile([128, 128], FP, name="sden",
                                tag="sdh" if hi else "sden",
                                bufs=2 if hi else 5)
                    nc.vector.reduce_sum(
                        d[:], e[:].rearrange("p (b s) -> p b s", s=4),
                        axis=AX.X)
                    if hi:
                        nc.vector.tensor_tensor(st["den"][c][:],
                                                st["den"][c][:], d[:],
                                                ALU.add)
                        st["e_hi"][c] = e
                    else:
                        st["den"][c] = d
                        st["e_lo"][c] = e

            pending = []

            def drain_kv_chunk(jj, hi, c, ps_c, st):
                """Vector part of a kv drain; transpose+copy deferred."""
                e = st["e_hi" if hi else "e_lo"][c]
                p = pp.tile([128, 512], BF, tag="pt", name="pt")
                nc.vector.tensor_tensor(p[:], e[:], ps_c[:], ALU.mult)
                n = sp.tile([128, 128], FP, name="snum",
                            tag="snh" if hi else "snum",
                            bufs=2 if hi else 5)
                nc.vector.reduce_sum(
                    n[:], p[:].rearrange("p (b s) -> p b s", s=4), axis=AX.X)
                if hi:
                    nc.vector.tensor_tensor(st["num"][c][:], st["num"][c][:],
                                            n[:], ALU.add)
                    inv = sp.tile([128, 128], FP, tag="sinv", name="sinv",
                                  bufs=2)
                    nc.vector.reciprocal(inv[:], st["den"][c][:])
                    pooled = sp.tile([128, 128], FP, tag="spool",
                                     name="spool", bufs=6)
                    nc.vector.tensor_tensor(pooled[:], st["num"][c][:],
                                            inv[:], ALU.mult)
                    pending.append((jj, c, pooled))
                else:
                    st["num"][c] = n

            def flush_pending():
                # [channel, block] -> [block, channel]; runs when the
                # tensor engine reaches it, long after `pooled` is ready
                while pending:
                    jj, c, pooled = pending.pop(0)
                    trp = pmm.tile([128, 512], FP, tag="ps", name="trp")
                    nc.tensor.transpose(trp[:, 0:128], pooled[:], ident[:])
                    nc.scalar.copy(ys[c][:, 128 * jj: 128 * jj + 128],
                                   trp[:, 0:128])

            def epilogue(c):
                y = ys[c]
                # RMSNorm over the 512 channels
                sq = pp.tile([128, 512], BF, tag="pt", name="sq")
                nc.vector.tensor_tensor(sq[:], y[:], y[:], ALU.mult)
                ssum = sp.tile([128, 1], FP, tag="s1", name="ssum", bufs=3)
                nc.vector.reduce_sum(ssum[:], sq[:], axis=AX.X)
                rs = sp.tile([128, 1], FP, tag="s1", name="rs", bufs=3)
                nc.scalar.activation(rs[:], ssum[:], ACTF.Sqrt,
                                     bias=eps_sb[:], scale=1.0 / D)
                inv_rs = sp.tile([128, 1], FP, tag="s1", name="invrs", bufs=3)
                nc.vector.reciprocal(inv_rs[:], rs[:])
                nc.vector.tensor_scalar_mul(y[:], y[:], inv_rs[:])
                nc.vector.tensor_tensor(y[:], y[:], norm_sb[:], ALU.mult)
                # RoPE on the last 64 channels
                yr = y[:, 448:512].rearrange("p (m two) -> p m two", two=2)
                a, b = yr[:, :, 0], yr[:, :, 1]
                t1 = sp.tile([128, 32], FP, tag="s1", name="t1", bufs=3)
                t2 = sp.tile([128, 32], FP, tag="s1", name="t2", bufs=3)
                t3 = sp.tile([128, 32], FP, tag="s1", name="t3", bufs=3)
                t4 = sp.tile([128, 32], FP, tag="s1", name="t4", bufs=3)
                nc.vector.tensor_tensor(t1[:], a, cos_sb[c][:], ALU.mult)
                nc.vector.tensor_tensor(t2[:], b, sin_sb[c][:], ALU.mult)
                nc.vector.tensor_tensor(t3[:], a, sin_sb[c][:], ALU.mult)
                nc.vector.tensor_tensor(t4[:], b, cos_sb[c][:], ALU.mult)
                nc.vector.tensor_tensor(a, t1[:], t2[:], ALU.subtract)
                nc.vector.tensor_tensor(b, t3[:], t4[:], ALU.add)
                yo = tp.tile([128, 512], FP, tag="tt", name="yo")
                nc.scalar.copy(yo[:], y[:])
                nc.sync.dma_start(out[128 * c: 128 * c + 128, :], yo[:])

            st_by_fam = {jj: {"e_lo": {}, "e_hi": {}, "den": {}, "num": {}}
                         for jj in range(4)}

            # phase A: the two score ocs of family 0, k-interleaved so the
            # x stream stays ahead
            pss8 = [pmm.tile([128, 512], FP, tag="ps", name="ps")
                    for _ in range(4)]
            pss12 = [pmm.tile([128, 512], FP, tag="ps", name="ps")
                     for _ in range(4)]
            for k in range(KBF):
                emit_mms(8, pss8, ks=[k], with_f8=False)
                emit_mms(12, pss12, ks=[k], with_f8=False)
            for kk in range(KP8):
                for oc, pss in ((8, pss8), (12, pss12)):
                    lhsT8 = w8_sb[oc][:, kk, :, :]
                    off = _oc_off(oc)
                    for c in range(4):
                        nc.tensor.matmul(
                            pss[c], lhsT=lhsT8,
                            rhs=x8ts[kk][:, :,
                                         off + 512 * c: off + 512 * c + 512],
                            start=False, stop=(kk == KP8 - 1),
                            perf_mode=DR, skip_group_check=True)
            load_w(OC_ORDER[2], nc.sync)
            drain_score(0, False, pss8, st_by_fam[0])
            load_w(OC_ORDER[3], nc.scalar)
            drain_score(0, True, pss12, st_by_fam[0])

            # steady state: one oc at a time, 4+4 psum ping-pong
            for i in range(2, 16):
                oc = OC_ORDER[i]
                jj = oc % 4
                if i + 2 < 16:
                    load_w(OC_ORDER[i + 2], nc.sync if i % 2 == 0
                           else nc.scalar)
                st = st_by_fam[jj]
                if i == 15:
                    # last oc: k-major like the rest; serial drain tail
                    pss = mm_oc(oc)
                    for c in range(4):
                        drain_kv_chunk(jj, True, c, pss[c], st)
                        flush_pending()
                        epilogue(c)
                else:
                    pss = mm_oc(oc)
                    flush_pending()
                    if oc >= 12:
                        drain_score(jj, True, pss, st)
                    elif oc >= 8:
                        drain_score(jj, False, pss, st)
                    else:
                        for c in range(4):
                            drain_kv_chunk(jj, oc >= 4, c, pss[c], st)

    nc.finalize()
    return nc


_PROGRAM = None


def _get_program() -> bass.Bass:
    global _PROGRAM
    if _PROGRAM is None:
        _PROGRAM = _build_program()
    return _PROGRAM


def host_prep(inputs) -> list[dict]:
    x = np.asarray(inputs["x"], dtype=np.float32)
    wkv = np.asarray(inputs["wkv_w"], dtype=np.float32)
    wg = np.asarray(inputs["wgate_w"], dtype=np.float32)
    ape = np.asarray(inputs["ape"], dtype=np.float32)
    norm_w = np.asarray(inputs["norm_w"], dtype=np.float32)
    cos = np.asarray(inputs["cos"], dtype=np.float32)
    sin = np.asarray(inputs["sin"], dtype=np.float32)

    # bf16 weights: kv rows as-is; gate rows pre-scaled by SCALE
    W_cat = np.concatenate([wkv, wg * SCALE], axis=0)  # [2048, 4096]
    # w_prep[oc, ki, kt, m] = W_cat[128*oc + m, 128*kt + ki]
    w_prep = np.ascontiguousarray(
        W_cat.reshape(16, 128, 32, 128).transpose(0, 3, 2, 1)
        .astype(ml_dtypes.bfloat16))
    # fp8 gate weights for K dims [128*KBF, 4096): [j, p, kk, t, m]
    wg8 = (wg[:, 128 * KBF:] * SW).astype(ml_dtypes.float8_e4m3fn)
    w8_prep = np.ascontiguousarray(
        wg8.reshape(8, 128, KP8, 2, 128).transpose(0, 4, 2, 3, 1))
    ape_prep = np.ascontiguousarray(ape.T.reshape(8, 128, 4) * SCALE)
    cos_s = np.ascontiguousarray(cos[::RATIO][: S // RATIO])   # [1024, 32]
    sin_s = np.ascontiguousarray(sin[::RATIO][: S // RATIO])
    norm_b = np.ascontiguousarray(
        np.broadcast_to(norm_w[None, :], (128, 512))
        .astype(ml_dtypes.bfloat16))
    fix_neg = np.full((128, 4), NEG * SCALE, np.float32)
    fix_zero = np.zeros((128, 4), np.float32)

    in_maps = []
    for core in range(NCORES):
        b, half = core // 2, core % 2
        xb = x[b]
        if half == 0:
            xs = np.concatenate(
                [np.zeros((4, DIM), np.float32), xb[:TOK]], axis=0)
        else:
            xs = xb[TOK - 4: 2 * TOK]
        xT = np.ascontiguousarray(xs.T)                     # [4096, XW]
        xT_bf = np.ascontiguousarray(
            xT.astype(ml_dtypes.bfloat16).reshape(32, 128, XW))
        # fp8 copy of the high-K dims: [kk, p, t, col]
        x8_prep = np.ascontiguousarray(
            (xT[128 * KBF:] * SX).astype(ml_dtypes.float8_e4m3fn)
            .reshape(KP8, 2, 128, XW).transpose(0, 2, 1, 3))
        in_maps.append(dict(
            xt=xT_bf,
            x8=x8_prep,
            w=w_prep,
            w8=w8_prep,
            ape=ape_prep,
            cosp=np.ascontiguousarray(cos_s[half * 512: half * 512 + 512]),
            sinp=np.ascontiguousarray(sin_s[half * 512: half * 512 + 512]),
            normb=norm_b,
            scfix=(fix_neg if half == 0 else fix_zero),
        ))
    return in_maps


def assemble(results) -> np.ndarray:
    full = np.zeros((B, S // RATIO, D), np.float32)
    for core in range(NCORES):
        b, half = core // 2, core % 2
        full[b, half * 512: half * 512 + 512] = results[core]["out"]
    return full


def kernel(**inputs) -> np.ndarray:
    import os
    nc = _get_program()
    in_maps = host_prep(inputs)
    # force the plain execute path: a stray BASS_TRACE would route through
    # profiling hooks this environment may not have
    prev = os.environ.get("BASS_NEVER_TRACE")
    os.environ["BASS_NEVER_TRACE"] = "1"
    try:
        res = run_bass_kernel_spmd(nc, in_maps, list(range(NCORES)))
    finally:
        if prev is None:
            os.environ.pop("BASS_NEVER_TRACE", None)
        else:
            os.environ["BASS_NEVER_TRACE"] = prev
    return assemble(res.results)


# revision 9
# speedup vs baseline: 1.0183x; 1.0183x over previous
"""Trainium2 Bass kernel for the Compressor module (sparse-attention KV
compression): fused kv/score projections -> overlapped softmax pooling ->
RMSNorm -> RoPE.

Sharding: data-parallel over (batch x seq-half) across 8 cores. Each core
processes 2048 tokens of one batch with a 4-token halo at the front, so no
collectives are needed. Weights are replicated.

Layout: matmuls compute out.T = W @ x.T ([channel, token]). x.T is resident
in SBUF (32 bf16 k-rows of [128, 2052]); weight tiles stream through a
3-deep pool, each used for 4 moving matmuls (one per 512-token chunk), so
LdWeights is amortized 4x. One output-channel tile (4 psum banks) is in
flight at a time, ping-ponging with the previous tile's drain. The
"overlap" halves come from a 4-token shifted matmul window (lo channels
cols [0,2048), hi cols [4,2052)), which makes every softmax-pool group of
4 column-aligned.

Precision: kv matmuls all bf16 (~0.3% err). Score matmuls are split-K:
first 2560 dims bf16 with weights pre-scaled by 2048, last 1536 dims in
fp8-e4m3 DoubleRow perf mode (2 K-rows/cycle) with x*8 / w*256 scaling --
the psum ends up at 2048x scale, undone for free by the Exp activation's
scale argument. Pool/epilogue intermediates in bf16 where harmless.
Predicted rel err ~1.5e-2 vs the 2e-2 budget (validated numerically
against the real inputs; the numeric sim matches hardware to 6 digits).

Scheduling: PE-array transposes are deferred until after the NEXT oc's
matmuls are emitted so the tensor queue never waits on the vector drain
chain (which would drop the PE to half-clock pstate). The last oc runs
chunk-major with drains one chunk behind for the same reason. Weight DMAs
ride the fast sync/scalar queues ahead of the x stream; a tiny warm-up
matmul gated on x-row 3 delays the tensor start just enough that the x
stream stays ahead of consumption.
"""

import numpy as np
import ml_dtypes

import concourse.bass as bass
import concourse.mybir as mybir
from concourse import bacc
from concourse.tile import TileContext
from concourse.masks import make_identity
from concourse.bass_utils import run_bass_kernel_spmd

B, S, DIM = 4, 4096, 4096
D, RD, RATIO = 512, 64, 4
EPS = 1e-6
NCORES = 8
TOK = 2048          # tokens per core
XW = TOK + 4        # x window width (4-token halo at the front)
KBF = 22            # bf16 k-tiles for score ocs (kv uses all 32)
KP8 = 5             # fp8 DoubleRow k-pair-tiles for score ocs
SCALE = 2048.0      # score psum scale (SX * SW)
SX, SW = 8.0, 256.0
NEG = -1.0e30
FP = mybir.dt.float32
BF = mybir.dt.bfloat16
F8 = mybir.dt.float8e4
DR = mybir.MatmulPerfMode.DoubleRow
AX = mybir.AxisListType
ALU = mybir.AluOpType
ACTF = mybir.ActivationFunctionType

# oc tiles: 0..7 = kv channels (lo 0..3, hi 4..7), 8..15 = gate/score
# channels (lo 8..11, hi 12..15). Family jj uses {jj, 4+jj, 8+jj, 12+jj}.
OC_ORDER = [8, 12, 0, 4,
            9, 13, 1, 5,
            10, 14, 2, 6,
            11, 15, 3, 7]


def _oc_off(oc: int) -> int:
    """Token-window offset: lo channels read cols [0,2048), hi [4,2052)."""
    return 4 if (oc // 4) % 2 == 1 else 0


def _build_program() -> bass.Bass:
    nc = bacc.Bacc("TRN2", target_bir_lowering=False, debug=False)

    xt = nc.dram_tensor("xt", [32, 128, XW], BF, kind="ExternalInput").ap()
    x8 = nc.dram_tensor("x8", [KP8, 128, 2, XW], F8,
                        kind="ExternalInput").ap()
    w = nc.dram_tensor("w", [16, 128, 32, 128], BF,
                       kind="ExternalInput").ap()
    w8 = nc.dram_tensor("w8", [8, 128, KP8, 2, 128], F8,
                        kind="ExternalInput").ap()
    ape = nc.dram_tensor("ape", [8, 128, 4], FP, kind="ExternalInput").ap()
    cosp = nc.dram_tensor("cosp", [512, 32], FP, kind="ExternalInput").ap()
    sinp = nc.dram_tensor("sinp", [512, 32], FP, kind="ExternalInput").ap()
    normb = nc.dram_tensor("normb", [128, 512], BF, kind="ExternalInput").ap()
    scfix = nc.dram_tensor("scfix", [128, 4], FP, kind="ExternalInput").ap()
    out = nc.dram_tensor("out", [512, 512], FP, kind="ExternalOutput").ap()

    with TileContext(nc) as tc:
        with (
            tc.tile_pool(name="const", bufs=1) as constp,
            tc.tile_pool(name="xp", bufs=32) as xp,
            tc.tile_pool(name="x8p", bufs=KP8) as x8p,
            tc.tile_pool(name="wp", bufs=3) as wp,
            tc.tile_pool(name="w8p", bufs=2) as w8p,
            tc.tile_pool(name="ep", bufs=8) as ep,
            tc.tile_pool(name="tp", bufs=2) as tp,
            tc.tile_pool(name="pp", bufs=2) as pp,
            tc.tile_pool(name="sp", bufs=6) as sp,
            tc.tile_pool(name="yp", bufs=4) as yp,
            tc.tile_pool(name="pmm", bufs=8, space="PSUM") as pmm,
        ):
            ident = constp.tile([128, 128], FP)
            make_identity(nc, ident)
            norm_sb = constp.tile([128, 512], BF)
            nc.gpsimd.dma_start(norm_sb, normb)
            fix_sb = constp.tile([128, 4], FP)
            nc.gpsimd.dma_start(fix_sb, scfix)
            ape_sb = constp.tile([128, 8, 4], FP)
            nc.gpsimd.dma_start(ape_sb, ape.rearrange("j p q -> p j q"))
            eps_sb = constp.tile([128, 1], FP)
            nc.gpsimd.memset(eps_sb[:], EPS)
            cos_sb, sin_sb = {}, {}
            for c in range(4):
                t = constp.tile([128, 32], FP, name=f"cos{c}")
                nc.gpsimd.dma_start(t, cosp[128 * c: 128 * c + 128, :])
                cos_sb[c] = t
                t = constp.tile([128, 32], FP, name=f"sin{c}")
                nc.gpsimd.dma_start(t, sinp[128 * c: 128 * c + 128, :])
                sin_sb[c] = t

            w_sb, w8_sb = {}, {}

            def load_w(oc, eng):
                t = wp.tile([128, 32, 128], BF, tag="wt", name="wt")
                if oc < 8:
                    eng.dma_start(t, w[oc])
                else:
                    eng.dma_start(t[:, 0:KBF, :], w[oc, :, 0:KBF, :])
                    t8 = w8p.tile([128, KP8, 2, 128], F8, tag="w8",
                                  name="w8t")
                    eng.dma_start(t8, w8[oc - 8])
                    w8_sb[oc] = t8
                w_sb[oc] = t

            # weight tiles for the first two ocs at the head of the two
            # fast DMA queues, then the x stream (fp8 rows first: small,
            # and needed mid-phase-A)
            load_w(OC_ORDER[0], nc.sync)
            load_w(OC_ORDER[1], nc.scalar)

            x8ts = []
            for kk in range(KP8):
                t = x8p.tile([128, 2, XW], F8, tag="x8t", name="x8tile")
                eng = nc.sync if kk % 2 == 0 else nc.scalar
                eng.dma_start(t, x8[kk])
                x8ts.append(t)
            xts = []
            for k in range(32):
                t = xp.tile([128, XW], BF, tag="xt", name="xtile")
                eng = nc.sync if k % 2 == 0 else nc.scalar
                eng.dma_start(t, xt[k])
                xts.append(t)

            # warm-up: tiny matmul gated on x-row 3 so the tensor engine
            # wakes only once the x stream has a head start
            warm = pmm.tile([128, 512], FP, tag="ps", name="warm")
            nc.tensor.matmul(warm[0:8, 0:8], lhsT=w_sb[8][:, 0, 0:8],
                             rhs=xts[3][:, 0:8], start=True, stop=True,
                             skip_group_check=True)

            ys = {}
            for c in range(4):
                ys[c] = yp.tile([128, 512], BF, tag="y", name="y")

            def emit_mms(oc, pss, ks=None, chunks=(0, 1, 2, 3),
                         with_f8=True):
                """Matmul stream for one oc into per-chunk psums."""
                off = _oc_off(oc)
                is_score = oc >= 8
                nk = KBF if is_score else 32
                for k in (range(nk) if ks is None else ks):
                    lhsT = w_sb[oc][:, k, :]
                    for c in chunks:
                        nc.tensor.matmul(
                            pss[c], lhsT=lhsT,
                            rhs=xts[k][:, off + 512 * c: off + 512 * c + 512],
                            start=(k == 0),
                            stop=(not is_score and k == 31),
                            skip_group_check=True)
                if is_score and with_f8:
                    for kk in range(KP8):
                        lhsT8 = w8_sb[oc][:, kk, :, :]
                        for c in chunks:
                            nc.tensor.matmul(
                                pss[c], lhsT=lhsT8,
                                rhs=x8ts[kk][:, :,
                                             off + 512 * c:
                                             off + 512 * c + 512],
                                start=False, stop=(kk == KP8 - 1),
                                perf_mode=DR, skip_group_check=True)

            def mm_oc(oc):
                pss = [pmm.tile([128, 512], FP, tag="ps", name="ps")
                       for _ in range(4)]
                emit_mms(oc, pss)
                return pss

            def drain_score(jj, hi, pss, st):
                ap_idx = 4 + jj if hi else jj
                for c in range(4):
                    t = tp.tile([128, 512], FP, tag="tt", name="tt")
                    nc.vector.tensor_tensor(
                        t[:].rearrange("p (b s) -> p b s", s=4),
                        pss[c][:].rearrange("p (b s) -> p b s", s=4),
                        ape_sb[:, ap_idx, None, :].to_broadcast((128, 128, 4)),
                        ALU.add)
                    if (not hi) and c == 0:
                        # first block of the shard: -inf for the missing
                        # previous block (no-op on odd cores)
                        nc.vector.tensor_tensor(t[:, 0:4], t[:, 0:4],
                                                fix_sb[:], ALU.add)
                    e = ep.tile([128, 512], BF, tag="et", name="et")
                    nc.scalar.activation(e[:], t[:], ACTF.Exp,
                                         scale=1.0 / SCALE)
                    d = sp.tile([128, 128], FP, name="sden",
                                tag="sdh" if hi else "sden",
                                bufs=2 if hi else 5)
                    nc.vector.reduce_sum(
                        d[:], e[:].rearrange("p (b s) -> p b s", s=4),
                        axis=AX.X)
                    if hi:
                        nc.vector.tensor_tensor(st["den"][c][:],
                                                st["den"][c][:], d[:],
                                                ALU.add)
                        st["e_hi"][c] = e
                    else:
                        st["den"][c] = d
                        st["e_lo"][c] = e

            pending = []

            def drain_kv_chunk(jj, hi, c, ps_c, st):
                """Vector part of a kv drain; transpose+copy deferred."""
                e = st["e_hi" if hi else "e_lo"][c]
                p = pp.tile([128, 512], BF, tag="pt", name="pt")
                nc.vector.tensor_tensor(p[:], e[:], ps_c[:], ALU.mult)
                n = sp.tile([128, 128], FP, name="snum",
                            tag="snh" if hi else "snum",
                            bufs=2 if hi else 5)
                nc.vector.reduce_sum(
                    n[:], p[:].rearrange("p (b s) -> p b s", s=4), axis=AX.X)
                if hi:
                    nc.vector.tensor_tensor(st["num"][c][:], st["num"][c][:],
                                            n[:], ALU.add)
                    inv = sp.tile([128, 128], FP, tag="sinv", name="sinv",
                                  bufs=2)
                    nc.vector.reciprocal(inv[:], st["den"][c][:])
                    pooled = sp.tile([128, 128], FP, tag="spool",
                                     name="spool", bufs=6)
                    nc.vector.tensor_tensor(pooled[:], st["num"][c][:],
                                            inv[:], ALU.mult)
                    pending.append((jj, c, pooled))
                else:
                    st["num"][c] = n

            def flush_pending():
                # [channel, block] -> [block, channel]; runs when the
                # tensor engine reaches it, long after `pooled` is ready
                while pending:
                    jj, c, pooled = pending.pop(0)
                    trp = pmm.tile([128, 512], FP, tag="ps", name="trp")
                    nc.tensor.transpose(trp[:, 0:128], pooled[:], ident[:])
                    nc.scalar.copy(ys[c][:, 128 * jj: 128 * jj + 128],
                                   trp[:, 0:128])

            def epilogue(c):
                y = ys[c]
                # RMSNorm over the 512 channels
                sq = pp.tile([128, 512], BF, tag="pt", name="sq")
                nc.vector.tensor_tensor(sq[:], y[:], y[:], ALU.mult)
                ssum = sp.tile([128, 1], FP, tag="s1", name="ssum", bufs=3)
                nc.vector.reduce_sum(ssum[:], sq[:], axis=AX.X)
                rs = sp.tile([128, 1], FP, tag="s1", name="rs", bufs=3)
                nc.scalar.activation(rs[:], ssum[:], ACTF.Sqrt,
                                     bias=eps_sb[:], scale=1.0 / D)
                inv_rs = sp.tile([128, 1], FP, tag="s1", name="invrs", bufs=3)
                nc.vector.reciprocal(inv_rs[:], rs[:])
                nc.vector.tensor_scalar_mul(y[:], y[:], inv_rs[:])
                nc.vector.tensor_tensor(y[:], y[:], norm_sb[:], ALU.mult)
                # RoPE on the last 64 channels
                yr = y[:, 448:512].rearrange("p (m two) -> p m two", two=2)
                a, b = yr[:, :, 0], yr[:, :, 1]
                t1 = sp.tile([128, 32], FP, tag="s1", name="t1", bufs=3)
                t2 = sp.tile([128, 32], FP, tag="s1", name="t2", bufs=3)
                t3 = sp.tile([128, 32], FP, tag="s1", name="t3", bufs=3)
                t4 = sp.tile([128, 32], FP, tag="s1", name="t4", bufs=3)
                nc.vector.tensor_tensor(t1[:], a, cos_sb[c][:], ALU.mult)
                nc.vector.tensor_tensor(t2[:], b, sin_sb[c][:], ALU.mult)
                nc.vector.tensor_tensor(t3[:], a, sin_sb[c][:], ALU.mult)
                nc.vector.tensor_tensor(t4[:], b, cos_sb[c][:], ALU.mult)
                nc.vector.tensor_tensor(a, t1[:], t2[:], ALU.subtract)
                nc.vector.tensor_tensor(b, t3[:], t4[:], ALU.add)
                yo = tp.tile([128, 512], FP, tag="tt", name="yo")
                nc.scalar.copy(yo[:], y[:])
                nc.sync.dma_start(out[128 * c: 128 * c + 128, :], yo[:])

            st_by_fam = {jj: {"e_lo": {}, "e_hi": {}, "den": {}, "num": {}}
                         for jj in range(4)}

            # phase A: the two score ocs of family 0, k-interleaved so the
            # x stream stays ahead
            pss8 = [pmm.tile([128, 512], FP, tag="ps", name="ps")
                    for _ in range(4)]
            pss12 = [pmm.tile([128, 512], FP, tag="ps", name="ps")
                     for _ in range(4)]
            for k in range(KBF):
                emit_mms(8, pss8, ks=[k], with_f8=False)
                emit_mms(12, pss12, ks=[k], with_f8=False)
            for kk in range(KP8):
                for oc, pss in ((8, pss8), (12, pss12)):
                    lhsT8 = w8_sb[oc][:, kk, :, :]
                    off = _oc_off(oc)
                    for c in range(4):
                        nc.tensor.matmul(
                            pss[c], lhsT=lhsT8,
                            rhs=x8ts[kk][:, :,
                                         off + 512 * c: off + 512 * c + 512],
                            start=False, stop=(kk == KP8 - 1),
                            perf_mode=DR, skip_group_check=True)
            load_w(OC_ORDER[2], nc.sync)
            drain_score(0, False, pss8, st_by_fam[0])
            load_w(OC_ORDER[3], nc.scalar)
            drain_score(0, True, pss12, st_by_fam[0])

            # steady state: one oc at a time, 4+4 psum ping-pong
            for i in range(2, 16):
                oc = OC_ORDER[i]
                jj = oc % 4
                if i + 2 < 16:
                    load_w(OC_ORDER[i + 2], nc.sync if i % 2 == 0
                           else nc.scalar)
                st = st_by_fam[jj]
                if i == 15:
                    # last oc: chunk-pair-major so the first pair's drains
                    # and epilogues overlap the second pair's matmuls
                    pss = [pmm.tile([128, 512], FP, tag="ps", name="ps")
                           for _ in range(4)]
                    emit_mms(oc, pss, chunks=(0, 1))
                    drain_kv_chunk(jj, True, 0, pss[0], st)
                    drain_kv_chunk(jj, True, 1, pss[1], st)
                    emit_mms(oc, pss, chunks=(2, 3))
                    flush_pending()
                    epilogue(0)
                    epilogue(1)
                    drain_kv_chunk(jj, True, 2, pss[2], st)
                    drain_kv_chunk(jj, True, 3, pss[3], st)
                    flush_pending()
                    epilogue(2)
                    epilogue(3)
                else:
                    pss = mm_oc(oc)
                    flush_pending()
                    if oc >= 12:
                        drain_score(jj, True, pss, st)
                    elif oc >= 8:
                        drain_score(jj, False, pss, st)
                    else:
                        for c in range(4):
                            drain_kv_chunk(jj, oc >= 4, c, pss[c], st)

    nc.finalize()
    return nc


_PROGRAM = None


def _get_program() -> bass.Bass:
    global _PROGRAM
    if _PROGRAM is None:
        _PROGRAM = _build_program()
    return _PROGRAM


def host_prep(inputs) -> list[dict]:
    x = np.asarray(inputs["x"], dtype=np.float32)
    wkv = np.asarray(inputs["wkv_w"], dtype=np.float32)
    wg = np.asarray(inputs["wgate_w"], dtype=np.float32)
    ape = np.asarray(inputs["ape"], dtype=np.float32)
    norm_w = np.asarray(inputs["norm_w"], dtype=np.float32)
    cos = np.asarray(inputs["cos"], dtype=np.float32)
    sin = np.asarray(inputs["sin"], dtype=np.float32)

    # bf16 weights: kv rows as-is; gate rows pre-scaled by SCALE
    W_cat = np.concatenate([wkv, wg * SCALE], axis=0)  # [2048, 4096]
    # w_prep[oc, ki, kt, m] = W_cat[128*oc + m, 128*kt + ki]
    w_prep = np.ascontiguousarray(
        W_cat.reshape(16, 128, 32, 128).transpose(0, 3, 2, 1)
        .astype(ml_dtypes.bfloat16))
    # fp8 gate weights for K dims [128*KBF, 4096): [j, p, kk, t, m]
    wg8 = (wg[:, 128 * KBF:] * SW).astype(ml_dtypes.float8_e4m3fn)
    w8_prep = np.ascontiguousarray(
        wg8.reshape(8, 128, KP8, 2, 128).transpose(0, 4, 2, 3, 1))
    ape_prep = np.ascontiguousarray(ape.T.reshape(8, 128, 4) * SCALE)
    cos_s = np.ascontiguousarray(cos[::RATIO][: S // RATIO])   # [1024, 32]
    sin_s = np.ascontiguousarray(sin[::RATIO][: S // RATIO])
    norm_b = np.ascontiguousarray(
        np.broadcast_to(norm_w[None, :], (128, 512))
        .astype(ml_dtypes.bfloat16))
    fix_neg = np.full((128, 4), NEG * SCALE, np.float32)
    fix_zero = np.zeros((128, 4), np.float32)

    in_maps = []
    for core in range(NCORES):
        b, half = core // 2, core % 2
        xb = x[b]
        if half == 0:
            xs = np.concatenate(
                [np.zeros((4, DIM), np.float32), xb[:TOK]], axis=0)
        else:
            xs = xb[TOK - 4: 2 * TOK]
        xT = np.ascontiguousarray(xs.T)                     # [4096, XW]
        xT_bf = np.ascontiguousarray(
            xT.astype(ml_dtypes.bfloat16).reshape(32, 128, XW))
        # fp8 copy of the high-K dims: [kk, p, t, col]
        x8_prep = np.ascontiguousarray(
            (xT[128 * KBF:] * SX).astype(ml_dtypes.float8_e4m3fn)
            .reshape(KP8, 2, 128, XW).transpose(0, 2, 1, 3))
        in_maps.append(dict(
            xt=xT_bf,
            x8=x8_prep,
            w=w_prep,
            w8=w8_prep,
            ape=ape_prep,
            cosp=np.ascontiguousarray(cos_s[half * 512: half * 512 + 512]),
            sinp=np.ascontiguousarray(sin_s[half * 512: half * 512 + 512]),
            normb=norm_b,
            scfix=(fix_neg if half == 0 else fix_zero),
        ))
    return in_maps


def assemble(results) -> np.ndarray:
    full = np.zeros((B, S // RATIO, D), np.float32)
    for core in range(NCORES):
        b, half = core // 2, core % 2
        full[b, half * 512: half * 512 + 512] = results[core]["out"]
    return full


def kernel(**inputs) -> np.ndarray:
    import os
    nc = _get_program()
    in_maps = host_prep(inputs)
    # force the plain execute path: a stray BASS_TRACE would route through
    # profiling hooks this environment may not have
    prev = os.environ.get("BASS_NEVER_TRACE")
    os.environ["BASS_NEVER_TRACE"] = "1"
    try:
        res = run_bass_kernel_spmd(nc, in_maps, list(range(NCORES)))
    finally:
        if prev is None:
            os.environ.pop("BASS_NEVER_TRACE", None)
        else:
            os.environ["BASS_NEVER_TRACE"] = prev
    return assemble(res.results)


# revision 10
# speedup vs baseline: 1.0333x; 1.0148x over previous
"""Trainium2 Bass kernel for the Compressor module (sparse-attention KV
compression): fused kv/score projections -> overlapped softmax pooling ->
RMSNorm -> RoPE.

Sharding: data-parallel over (batch x seq-half) across 8 cores. Each core
processes 2048 tokens of one batch with a 4-token halo at the front, so no
collectives are needed. Weights are replicated.

Layout: matmuls compute out.T = W @ x.T ([channel, token]). x.T is resident
in SBUF (32 bf16 k-rows of [128, 2052]); weight tiles stream through a
3-deep pool, each used for 4 moving matmuls (one per 512-token chunk), so
LdWeights is amortized 4x. One output-channel tile (4 psum banks) is in
flight at a time, ping-ponging with the previous tile's drain. The
"overlap" halves come from a 4-token shifted matmul window (lo channels
cols [0,2048), hi cols [4,2052)), which makes every softmax-pool group of
4 column-aligned.

Precision: kv matmuls all bf16 (~0.3% err). Score matmuls are split-K:
first 2560 dims bf16 with weights pre-scaled by 2048, last 1536 dims in
fp8-e4m3 DoubleRow perf mode (2 K-rows/cycle) with x*8 / w*256 scaling --
the psum ends up at 2048x scale, undone for free by the Exp activation's
scale argument. Pool/epilogue intermediates in bf16 where harmless.
Predicted rel err ~1.5e-2 vs the 2e-2 budget (validated numerically
against the real inputs; the numeric sim matches hardware to 6 digits).

Scheduling: PE-array transposes are deferred until after the NEXT oc's
matmuls are emitted so the tensor queue never waits on the vector drain
chain (which would drop the PE to half-clock pstate). The last oc runs
chunk-major with drains one chunk behind for the same reason. Weight DMAs
ride the fast sync/scalar queues ahead of the x stream; a tiny warm-up
matmul gated on x-row 3 delays the tensor start just enough that the x
stream stays ahead of consumption.
"""

import numpy as np
import ml_dtypes

import concourse.bass as bass
import concourse.mybir as mybir
from concourse import bacc
from concourse.tile import TileContext
from concourse.masks import make_identity
from concourse.bass_utils import run_bass_kernel_spmd

B, S, DIM = 4, 4096, 4096
D, RD, RATIO = 512, 64, 4
EPS = 1e-6
NCORES = 8
TOK = 2048          # tokens per core
XW = TOK + 4        # x window width (4-token halo at the front)
KBF = 22            # bf16 k-tiles for score ocs (kv uses all 32)
KP8 = 5             # fp8 DoubleRow k-pair-tiles for score ocs
SCALE = 2048.0      # score psum scale (SX * SW)
SX, SW = 8.0, 256.0
NEG = -1.0e30
FP = mybir.dt.float32
BF = mybir.dt.bfloat16
F8 = mybir.dt.float8e4
DR = mybir.MatmulPerfMode.DoubleRow
AX = mybir.AxisListType
ALU = mybir.AluOpType
ACTF = mybir.ActivationFunctionType

# oc tiles: 0..7 = kv channels (lo 0..3, hi 4..7), 8..15 = gate/score
# channels (lo 8..11, hi 12..15). Family jj uses {jj, 4+jj, 8+jj, 12+jj}.
OC_ORDER = [8, 12, 0, 4,
            9, 13, 1, 5,
            10, 14, 2, 6,
            11, 15, 3, 7]


def _oc_off(oc: int) -> int:
    """Token-window offset: lo channels read cols [0,2048), hi [4,2052)."""
    return 4 if (oc // 4) % 2 == 1 else 0


def _build_program() -> bass.Bass:
    nc = bacc.Bacc("TRN2", target_bir_lowering=False, debug=False)

    xt = nc.dram_tensor("xt", [32, 128, XW], BF, kind="ExternalInput").ap()
    x8 = nc.dram_tensor("x8", [KP8, 128, 2, XW], F8,
                        kind="ExternalInput").ap()
    w = nc.dram_tensor("w", [16, 128, 32, 128], BF,
                       kind="ExternalInput").ap()
    w8 = nc.dram_tensor("w8", [8, 128, KP8, 2, 128], F8,
                        kind="ExternalInput").ap()
    ape = nc.dram_tensor("ape", [8, 128, 4], FP, kind="ExternalInput").ap()
    cosp = nc.dram_tensor("cosp", [512, 32], FP, kind="ExternalInput").ap()
    sinp = nc.dram_tensor("sinp", [512, 32], FP, kind="ExternalInput").ap()
    normb = nc.dram_tensor("normb", [128, 512], BF, kind="ExternalInput").ap()
    scfix = nc.dram_tensor("scfix", [128, 4], FP, kind="ExternalInput").ap()
    out = nc.dram_tensor("out", [512, 512], FP, kind="ExternalOutput").ap()

    with TileContext(nc) as tc:
        with (
            tc.tile_pool(name="const", bufs=1) as constp,
            tc.tile_pool(name="xp", bufs=32) as xp,
            tc.tile_pool(name="x8p", bufs=KP8) as x8p,
            tc.tile_pool(name="wp", bufs=3) as wp,
            tc.tile_pool(name="w8p", bufs=2) as w8p,
            tc.tile_pool(name="ep", bufs=8) as ep,
            tc.tile_pool(name="tp", bufs=2) as tp,
            tc.tile_pool(name="pp", bufs=2) as pp,
            tc.tile_pool(name="sp", bufs=6) as sp,
            tc.tile_pool(name="yp", bufs=4) as yp,
            tc.tile_pool(name="pmm", bufs=8, space="PSUM") as pmm,
        ):
            ident = constp.tile([128, 128], FP)
            make_identity(nc, ident)
            norm_sb = constp.tile([128, 512], BF)
            nc.gpsimd.dma_start(norm_sb, normb)
            fix_sb = constp.tile([128, 4], FP)
            nc.gpsimd.dma_start(fix_sb, scfix)
            ape_sb = constp.tile([128, 8, 4], FP)
            nc.gpsimd.dma_start(ape_sb, ape.rearrange("j p q -> p j q"))
            eps_sb = constp.tile([128, 1], FP)
            nc.gpsimd.memset(eps_sb[:], EPS)
            cos_sb, sin_sb = {}, {}
            for c in range(4):
                t = constp.tile([128, 32], FP, name=f"cos{c}")
                nc.gpsimd.dma_start(t, cosp[128 * c: 128 * c + 128, :])
                cos_sb[c] = t
                t = constp.tile([128, 32], FP, name=f"sin{c}")
                nc.gpsimd.dma_start(t, sinp[128 * c: 128 * c + 128, :])
                sin_sb[c] = t

            w_sb, w8_sb = {}, {}

            def load_w(oc, eng):
                t = wp.tile([128, 32, 128], BF, tag="wt", name="wt")
                if oc < 8:
                    eng.dma_start(t, w[oc])
                else:
                    eng.dma_start(t[:, 0:KBF, :], w[oc, :, 0:KBF, :])
                    t8 = w8p.tile([128, KP8, 2, 128], F8, tag="w8",
                                  name="w8t")
                    eng.dma_start(t8, w8[oc - 8])
                    w8_sb[oc] = t8
                w_sb[oc] = t

            # weight tiles for the first two ocs at the head of the two
            # fast DMA queues, then the x stream (fp8 rows first: small,
            # and needed mid-phase-A)
            load_w(OC_ORDER[0], nc.sync)
            load_w(OC_ORDER[1], nc.scalar)

            x8ts = []
            for kk in range(KP8):
                t = x8p.tile([128, 2, XW], F8, tag="x8t", name="x8tile")
                eng = nc.sync if kk % 2 == 0 else nc.scalar
                eng.dma_start(t, x8[kk])
                x8ts.append(t)
            xts = []
            for k in range(32):
                t = xp.tile([128, XW], BF, tag="xt", name="xtile")
                eng = nc.sync if k % 2 == 0 else nc.scalar
                eng.dma_start(t, xt[k])
                xts.append(t)

            # warm-up: tiny matmul gated on x-row 3 so the tensor engine
            # wakes only once the x stream has a head start
            warm = pmm.tile([128, 512], FP, tag="ps", name="warm")
            nc.tensor.matmul(warm[0:8, 0:8], lhsT=w_sb[8][:, 0, 0:8],
                             rhs=xts[3][:, 0:8], start=True, stop=True,
                             skip_group_check=True)

            ys = {}
            for c in range(4):
                ys[c] = yp.tile([128, 512], BF, tag="y", name="y")

            def emit_mms(oc, pss, ks=None, chunks=(0, 1, 2, 3),
                         with_f8=True):
                """Matmul stream for one oc into per-chunk psums."""
                off = _oc_off(oc)
                is_score = oc >= 8
                nk = KBF if is_score else 32
                for k in (range(nk) if ks is None else ks):
                    lhsT = w_sb[oc][:, k, :]
                    for c in chunks:
                        nc.tensor.matmul(
                            pss[c], lhsT=lhsT,
                            rhs=xts[k][:, off + 512 * c: off + 512 * c + 512],
                            start=(k == 0),
                            stop=(not is_score and k == 31),
                            skip_group_check=True)
                if is_score and with_f8:
                    for kk in range(KP8):
                        lhsT8 = w8_sb[oc][:, kk, :, :]
                        for c in chunks:
                            nc.tensor.matmul(
                                pss[c], lhsT=lhsT8,
                                rhs=x8ts[kk][:, :,
                                             off + 512 * c:
                                             off + 512 * c + 512],
                                start=False, stop=(kk == KP8 - 1),
                                perf_mode=DR, skip_group_check=True)

            def mm_oc(oc):
                pss = [pmm.tile([128, 512], FP, tag="ps", name="ps")
                       for _ in range(4)]
                emit_mms(oc, pss)
                return pss

            def drain_score(jj, hi, pss, st):
                ap_idx = 4 + jj if hi else jj
                for c in range(4):
                    t = tp.tile([128, 512], FP, tag="tt", name="tt")
                    nc.vector.tensor_tensor(
                        t[:].rearrange("p (b s) -> p b s", s=4),
                        pss[c][:].rearrange("p (b s) -> p b s", s=4),
                        ape_sb[:, ap_idx, None, :].to_broadcast((128, 128, 4)),
                        ALU.add)
                    if (not hi) and c == 0:
                        # first block of the shard: -inf for the missing
                        # previous block (no-op on odd cores)
                        nc.vector.tensor_tensor(t[:, 0:4], t[:, 0:4],
                                                fix_sb[:], ALU.add)
                    e = ep.tile([128, 512], BF, tag="et", name="et")
                    nc.scalar.activation(e[:], t[:], ACTF.Exp,
                                         scale=1.0 / SCALE)
                    d = sp.tile([128, 128], FP, name="sden",
                                tag="sdh" if hi else "sden",
                                bufs=2 if hi else 5)
                    nc.vector.reduce_sum(
                        d[:], e[:].rearrange("p (b s) -> p b s", s=4),
                        axis=AX.X)
                    if hi:
                        nc.vector.tensor_tensor(st["den"][c][:],
                                                st["den"][c][:], d[:],
                                                ALU.add)
                        st["e_hi"][c] = e
                    else:
                        st["den"][c] = d
                        st["e_lo"][c] = e

            pending = []

            def drain_kv_chunk(jj, hi, c, ps_c, st):
                """Vector part of a kv drain; transpose+copy deferred."""
                e = st["e_hi" if hi else "e_lo"][c]
                p = pp.tile([128, 512], BF, tag="pt", name="pt")
                nc.vector.tensor_tensor(p[:], e[:], ps_c[:], ALU.mult)
                n = sp.tile([128, 128], FP, name="snum",
                            tag="snh" if hi else "snum",
                            bufs=2 if hi else 5)
                nc.vector.reduce_sum(
                    n[:], p[:].rearrange("p (b s) -> p b s", s=4), axis=AX.X)
                if hi:
                    nc.vector.tensor_tensor(st["num"][c][:], st["num"][c][:],
                                            n[:], ALU.add)
                    inv = sp.tile([128, 128], FP, tag="sinv", name="sinv",
                                  bufs=2)
                    nc.vector.reciprocal(inv[:], st["den"][c][:])
                    pooled = sp.tile([128, 128], FP, tag="spool",
                                     name="spool", bufs=6)
                    nc.vector.tensor_tensor(pooled[:], st["num"][c][:],
                                            inv[:], ALU.mult)
                    pending.append((jj, c, pooled))
                else:
                    st["num"][c] = n

            def flush_pending():
                # [channel, block] -> [block, channel]; runs when the
                # tensor engine reaches it, long after `pooled` is ready
                while pending:
                    jj, c, pooled = pending.pop(0)
                    trp = pmm.tile([128, 512], FP, tag="ps", name="trp")
                    nc.tensor.transpose(trp[:, 0:128], pooled[:], ident[:])
                    nc.scalar.copy(ys[c][:, 128 * jj: 128 * jj + 128],
                                   trp[:, 0:128])

            def epilogue(c):
                y = ys[c]
                # RMSNorm over the 512 channels
                sq = pp.tile([128, 512], BF, tag="pt", name="sq")
                nc.vector.tensor_tensor(sq[:], y[:], y[:], ALU.mult)
                ssum = sp.tile([128, 1], FP, tag="s1", name="ssum", bufs=3)
                nc.vector.reduce_sum(ssum[:], sq[:], axis=AX.X)
                rs = sp.tile([128, 1], FP, tag="s1", name="rs", bufs=3)
                nc.scalar.activation(rs[:], ssum[:], ACTF.Sqrt,
                                     bias=eps_sb[:], scale=1.0 / D)
                inv_rs = sp.tile([128, 1], FP, tag="s1", name="invrs", bufs=3)
                nc.vector.reciprocal(inv_rs[:], rs[:])
                nc.vector.tensor_scalar_mul(y[:], y[:], inv_rs[:])
                nc.vector.tensor_tensor(y[:], y[:], norm_sb[:], ALU.mult)
                # RoPE on the last 64 channels
                yr = y[:, 448:512].rearrange("p (m two) -> p m two", two=2)
                a, b = yr[:, :, 0], yr[:, :, 1]
                t1 = sp.tile([128, 32], FP, tag="s1", name="t1", bufs=3)
                t2 = sp.tile([128, 32], FP, tag="s1", name="t2", bufs=3)
                t3 = sp.tile([128, 32], FP, tag="s1", name="t3", bufs=3)
                t4 = sp.tile([128, 32], FP, tag="s1", name="t4", bufs=3)
                nc.vector.tensor_tensor(t1[:], a, cos_sb[c][:], ALU.mult)
                nc.vector.tensor_tensor(t2[:], b, sin_sb[c][:], ALU.mult)
                nc.vector.tensor_tensor(t3[:], a, sin_sb[c][:], ALU.mult)
                nc.vector.tensor_tensor(t4[:], b, cos_sb[c][:], ALU.mult)
                nc.vector.tensor_tensor(a, t1[:], t2[:], ALU.subtract)
                nc.vector.tensor_tensor(b, t3[:], t4[:], ALU.add)
                yo = tp.tile([128, 512], FP, tag="tt", name="yo")
                nc.scalar.copy(yo[:], y[:])
                nc.sync.dma_start(out[128 * c: 128 * c + 128, :], yo[:])

            st_by_fam = {jj: {"e_lo": {}, "e_hi": {}, "den": {}, "num": {}}
                         for jj in range(4)}

            # phase A: the two score ocs of family 0, k-interleaved so the
            # x stream stays ahead
            pss8 = [pmm.tile([128, 512], FP, tag="ps", name="ps")
                    for _ in range(4)]
            pss12 = [pmm.tile([128, 512], FP, tag="ps", name="ps")
                     for _ in range(4)]
            for k in range(KBF):
                emit_mms(8, pss8, ks=[k], with_f8=False)
                emit_mms(12, pss12, ks=[k], with_f8=False)
            for kk in range(KP8):
                for oc, pss in ((8, pss8), (12, pss12)):
                    lhsT8 = w8_sb[oc][:, kk, :, :]
                    off = _oc_off(oc)
                    for c in range(4):
                        nc.tensor.matmul(
                            pss[c], lhsT=lhsT8,
                            rhs=x8ts[kk][:, :,
                                         off + 512 * c: off + 512 * c + 512],
                            start=False, stop=(kk == KP8 - 1),
                            perf_mode=DR, skip_group_check=True)
            load_w(OC_ORDER[2], nc.sync)
            drain_score(0, False, pss8, st_by_fam[0])
            load_w(OC_ORDER[3], nc.scalar)
            drain_score(0, True, pss12, st_by_fam[0])

            # steady state: one oc at a time, 4+4 psum ping-pong
            for i in range(2, 16):
                oc = OC_ORDER[i]
                jj = oc % 4
                if i + 2 < 16:
                    load_w(OC_ORDER[i + 2], nc.sync if i % 2 == 0
                           else nc.scalar)
                st = st_by_fam[jj]
                if i == 15:
                    # last oc: chunk-major (pays ~160ns/matmul of exposed
                    # LdWeights, but every drain + epilogue except the very
                    # last overlaps the remaining matmul stream -- measured
                    # faster than k-major variants, whose bunched drain
                    # tail triggers the PE half-clock activity throttle)
                    for c in range(4):
                        ps_c = pmm.tile([128, 512], FP, tag="ps", name="ps")
                        emit_mms(oc, {c: ps_c}, chunks=(c,))
                        if c > 0:
                            flush_pending()
                            epilogue(c - 1)
                        drain_kv_chunk(jj, True, c, ps_c, st)
                    flush_pending()
                    epilogue(3)
                else:
                    pss = mm_oc(oc)
                    flush_pending()
                    if oc >= 12:
                        drain_score(jj, True, pss, st)
                    elif oc >= 8:
                        drain_score(jj, False, pss, st)
                    else:
                        for c in range(4):
                            drain_kv_chunk(jj, oc >= 4, c, pss[c], st)

    nc.finalize()
    return nc


_PROGRAM = None


def _get_program() -> bass.Bass:
    global _PROGRAM
    if _PROGRAM is None:
        _PROGRAM = _build_program()
    return _PROGRAM


def host_prep(inputs) -> list[dict]:
    x = np.asarray(inputs["x"], dtype=np.float32)
    wkv = np.asarray(inputs["wkv_w"], dtype=np.float32)
    wg = np.asarray(inputs["wgate_w"], dtype=np.float32)
    ape = np.asarray(inputs["ape"], dtype=np.float32)
    norm_w = np.asarray(inputs["norm_w"], dtype=np.float32)
    cos = np.asarray(inputs["cos"], dtype=np.float32)
    sin = np.asarray(inputs["sin"], dtype=np.float32)

    # bf16 weights: kv rows as-is; gate rows pre-scaled by SCALE
    W_cat = np.concatenate([wkv, wg * SCALE], axis=0)  # [2048, 4096]
    # w_prep[oc, ki, kt, m] = W_cat[128*oc + m, 128*kt + ki]
    w_prep = np.ascontiguousarray(
        W_cat.reshape(16, 128, 32, 128).transpose(0, 3, 2, 1)
        .astype(ml_dtypes.bfloat16))
    # fp8 gate weights for K dims [128*KBF, 4096): [j, p, kk, t, m]
    wg8 = (wg[:, 128 * KBF:] * SW).astype(ml_dtypes.float8_e4m3fn)
    w8_prep = np.ascontiguousarray(
        wg8.reshape(8, 128, KP8, 2, 128).transpose(0, 4, 2, 3, 1))
    ape_prep = np.ascontiguousarray(ape.T.reshape(8, 128, 4) * SCALE)
    cos_s = np.ascontiguousarray(cos[::RATIO][: S // RATIO])   # [1024, 32]
    sin_s = np.ascontiguousarray(sin[::RATIO][: S // RATIO])
    norm_b = np.ascontiguousarray(
        np.broadcast_to(norm_w[None, :], (128, 512))
        .astype(ml_dtypes.bfloat16))
    fix_neg = np.full((128, 4), NEG * SCALE, np.float32)
    fix_zero = np.zeros((128, 4), np.float32)

    in_maps = []
    for core in range(NCORES):
        b, half = core // 2, core % 2
        xb = x[b]
        if half == 0:
            xs = np.concatenate(
                [np.zeros((4, DIM), np.float32), xb[:TOK]], axis=0)
        else:
            xs = xb[TOK - 4: 2 * TOK]
        xT = np.ascontiguousarray(xs.T)                     # [4096, XW]
        xT_bf = np.ascontiguousarray(
            xT.astype(ml_dtypes.bfloat16).reshape(32, 128, XW))
        # fp8 copy of the high-K dims: [kk, p, t, col]
        x8_prep = np.ascontiguousarray(
            (xT[128 * KBF:] * SX).astype(ml_dtypes.float8_e4m3fn)
            .reshape(KP8, 2, 128, XW).transpose(0, 2, 1, 3))
        in_maps.append(dict(
            xt=xT_bf,
            x8=x8_prep,
            w=w_prep,
            w8=w8_prep,
            ape=ape_prep,
            cosp=np.ascontiguousarray(cos_s[half * 512: half * 512 + 512]),
            sinp=np.ascontiguousarray(sin_s[half * 512: half * 512 + 512]),
            normb=norm_b,
            scfix=(fix_neg if half == 0 else fix_zero),
        ))
    return in_maps


def assemble(results) -> np.ndarray:
    full = np.zeros((B, S // RATIO, D), np.float32)
    for core in range(NCORES):
        b, half = core // 2, core % 2
        full[b, half * 512: half * 512 + 512] = results[core]["out"]
    return full


def kernel(**inputs) -> np.ndarray:
    import os
    nc = _get_program()
    in_maps = host_prep(inputs)
    # force the plain execute path: a stray BASS_TRACE would route through
    # profiling hooks this environment may not have
    prev = os.environ.get("BASS_NEVER_TRACE")
    os.environ["BASS_NEVER_TRACE"] = "1"
    try:
        res = run_bass_kernel_spmd(nc, in_maps, list(range(NCORES)))
    finally:
        if prev is None:
            os.environ.pop("BASS_NEVER_TRACE", None)
        else:
            os.environ["BASS_NEVER_TRACE"] = prev
    return assemble(res.results)


# revision 11
# speedup vs baseline: 1.0585x; 1.0243x over previous
"""Trainium2 Bass kernel for the Compressor module (sparse-attention KV
compression): fused kv/score projections -> overlapped softmax pooling ->
RMSNorm -> RoPE.

Sharding: data-parallel over (batch x seq-half) across 8 cores. Each core
processes 2048 tokens of one batch with a 4-token halo at the front, so no
collectives are needed. Weights are replicated.

Layout: matmuls compute out.T = W @ x.T ([channel, token]). x.T is resident
in SBUF (32 bf16 k-rows of [128, 2052]); weight tiles stream through a
3-deep pool, each used for 4 moving matmuls (one per 512-token chunk), so
LdWeights is amortized 4x. One output-channel tile (4 psum banks) is in
flight at a time, ping-ponging with the previous tile's drain. The
"overlap" halves come from a 4-token shifted matmul window (lo channels
cols [0,2048), hi cols [4,2052)), which makes every softmax-pool group of
4 column-aligned.

Precision: kv matmuls all bf16 (~0.3% err). Score matmuls are split-K:
first 2560 dims bf16 with weights pre-scaled by 2048, last 1536 dims in
fp8-e4m3 DoubleRow perf mode (2 K-rows/cycle) with x*8 / w*256 scaling --
the psum ends up at 2048x scale, undone for free by the Exp activation's
scale argument. Pool/epilogue intermediates in bf16 where harmless.
Predicted rel err ~1.5e-2 vs the 2e-2 budget (validated numerically
against the real inputs; the numeric sim matches hardware to 6 digits).

Scheduling: PE-array transposes are deferred until after the NEXT oc's
matmuls are emitted so the tensor queue never waits on the vector drain
chain (which would drop the PE to half-clock pstate). The last oc runs
chunk-major with drains one chunk behind for the same reason. Weight DMAs
ride the fast sync/scalar queues ahead of the x stream; a tiny warm-up
matmul gated on x-row 3 delays the tensor start just enough that the x
stream stays ahead of consumption.
"""

import numpy as np
import ml_dtypes

import concourse.bass as bass
import concourse.mybir as mybir
from concourse import bacc
from concourse.tile import TileContext
from concourse.masks import make_identity
from concourse.bass_utils import run_bass_kernel_spmd

B, S, DIM = 4, 4096, 4096
D, RD, RATIO = 512, 64, 4
EPS = 1e-6
NCORES = 8
TOK = 2048          # tokens per core
XW = TOK + 4        # x window width (4-token halo at the front)
KBF = 22            # bf16 k-tiles for score ocs (kv uses all 32)
KP8 = 5             # fp8 DoubleRow k-pair-tiles for score ocs
KVBF = 30           # bf16 k-tiles for kv ocs (tiles 30,31 in fp8)
KVP = (KVBF - KBF) // 2   # x8 pair index covering tiles 30,31
SCALE = 2048.0      # score psum scale (SX * SW)
SX, SW = 8.0, 256.0
NEG = -1.0e30
FP = mybir.dt.float32
BF = mybir.dt.bfloat16
F8 = mybir.dt.float8e4
DR = mybir.MatmulPerfMode.DoubleRow
AX = mybir.AxisListType
ALU = mybir.AluOpType
ACTF = mybir.ActivationFunctionType

# oc tiles: 0..7 = kv channels (lo 0..3, hi 4..7), 8..15 = gate/score
# channels (lo 8..11, hi 12..15). Family jj uses {jj, 4+jj, 8+jj, 12+jj}.
OC_ORDER = [8, 12, 0, 4,
            9, 13, 1, 5,
            10, 14, 2, 6,
            11, 15, 3, 7]


def _oc_off(oc: int) -> int:
    """Token-window offset: lo channels read cols [0,2048), hi [4,2052)."""
    return 4 if (oc // 4) % 2 == 1 else 0


def _build_program() -> bass.Bass:
    nc = bacc.Bacc("TRN2", target_bir_lowering=False, debug=False)

    xt = nc.dram_tensor("xt", [KVBF, 128, XW], BF,
                    kind="ExternalInput").ap()
    x8 = nc.dram_tensor("x8", [KP8, 128, 2, XW], F8,
                        kind="ExternalInput").ap()
    w = nc.dram_tensor("w", [16, 128, 32, 128], BF,
                       kind="ExternalInput").ap()
    w8 = nc.dram_tensor("w8", [8, 128, KP8, 2, 128], F8,
                        kind="ExternalInput").ap()
    w8k = nc.dram_tensor("w8k", [8, 128, 2, 128], F8,
                         kind="ExternalInput").ap()
    ape = nc.dram_tensor("ape", [8, 128, 4], FP, kind="ExternalInput").ap()
    cosp = nc.dram_tensor("cosp", [512, 32], FP, kind="ExternalInput").ap()
    sinp = nc.dram_tensor("sinp", [512, 32], FP, kind="ExternalInput").ap()
    normb = nc.dram_tensor("normb", [128, 512], BF, kind="ExternalInput").ap()
    scfix = nc.dram_tensor("scfix", [128, 4], FP, kind="ExternalInput").ap()
    out = nc.dram_tensor("out", [512, 512], FP, kind="ExternalOutput").ap()

    with TileContext(nc) as tc:
        with (
            tc.tile_pool(name="const", bufs=1) as constp,
            tc.tile_pool(name="xp", bufs=KVBF) as xp,
            tc.tile_pool(name="x8p", bufs=KP8) as x8p,
            tc.tile_pool(name="wp", bufs=3) as wp,
            tc.tile_pool(name="w8p", bufs=2) as w8p,
            tc.tile_pool(name="ep", bufs=8) as ep,
            tc.tile_pool(name="tp", bufs=2) as tp,
            tc.tile_pool(name="pp", bufs=2) as pp,
            tc.tile_pool(name="sp", bufs=6) as sp,
            tc.tile_pool(name="yp", bufs=4) as yp,
            tc.tile_pool(name="pmm", bufs=8, space="PSUM") as pmm,
        ):
            ident = constp.tile([128, 128], FP)
            make_identity(nc, ident)
            norm_sb = constp.tile([128, 512], BF)
            nc.gpsimd.dma_start(norm_sb, normb)
            fix_sb = constp.tile([128, 4], FP)
            nc.gpsimd.dma_start(fix_sb, scfix)
            ape_sb = constp.tile([128, 8, 4], FP)
            nc.gpsimd.dma_start(ape_sb, ape.rearrange("j p q -> p j q"))
            eps_sb = constp.tile([128, 1], FP)
            nc.gpsimd.memset(eps_sb[:], EPS)
            cos_sb, sin_sb = {}, {}
            for c in range(4):
                t = constp.tile([128, 32], FP, name=f"cos{c}")
                nc.gpsimd.dma_start(t, cosp[128 * c: 128 * c + 128, :])
                cos_sb[c] = t
                t = constp.tile([128, 32], FP, name=f"sin{c}")
                nc.gpsimd.dma_start(t, sinp[128 * c: 128 * c + 128, :])
                sin_sb[c] = t

            w_sb, w8_sb = {}, {}

            def load_w(oc, eng):
                t = wp.tile([128, KVBF, 128], BF, tag="wt", name="wt")
                if oc < 8:
                    eng.dma_start(t, w[oc, :, 0:KVBF, :])
                    t8 = w8p.tile([128, 2, 128], F8, tag="w8k",
                                  name="w8kt", bufs=2)
                    eng.dma_start(t8, w8k[oc])
                    w8_sb[oc] = t8
                else:
                    eng.dma_start(t[:, 0:KBF, :], w[oc, :, 0:KBF, :])
                    t8 = w8p.tile([128, KP8, 2, 128], F8, tag="w8",
                                  name="w8t")
                    eng.dma_start(t8, w8[oc - 8])
                    w8_sb[oc] = t8
                w_sb[oc] = t

            # weight tiles for the first two ocs at the head of the two
            # fast DMA queues, then the x stream (fp8 rows first: small,
            # and needed mid-phase-A)
            load_w(OC_ORDER[0], nc.sync)
            load_w(OC_ORDER[1], nc.scalar)

            x8ts = []
            for kk in range(KP8):
                t = x8p.tile([128, 2, XW], F8, tag="x8t", name="x8tile")
                eng = nc.sync if kk % 2 == 0 else nc.scalar
                eng.dma_start(t, x8[kk])
                x8ts.append(t)
            xts = []
            for k in range(KVBF):
                t = xp.tile([128, XW], BF, tag="xt", name="xtile")
                eng = nc.sync if k % 2 == 0 else nc.scalar
                eng.dma_start(t, xt[k])
                xts.append(t)

            # warm-up: tiny matmul gated on x-row 3 so the tensor engine
            # wakes only once the x stream has a head start
            warm = pmm.tile([128, 512], FP, tag="ps", name="warm")
            nc.tensor.matmul(warm[0:8, 0:8], lhsT=w_sb[8][:, 0, 0:8],
                             rhs=xts[3][:, 0:8], start=True, stop=True,
                             skip_group_check=True)

            ys = {}
            for c in range(4):
                ys[c] = yp.tile([128, 512], BF, tag="y", name="y")

            def emit_mms(oc, pss, ks=None, chunks=(0, 1, 2, 3),
                         with_f8=True):
                """Matmul stream for one oc into per-chunk psums."""
                off = _oc_off(oc)
                is_score = oc >= 8
                nk = KBF if is_score else KVBF
                for k in (range(nk) if ks is None else ks):
                    lhsT = w_sb[oc][:, k, :]
                    for c in chunks:
                        nc.tensor.matmul(
                            pss[c], lhsT=lhsT,
                            rhs=xts[k][:, off + 512 * c: off + 512 * c + 512],
                            start=(k == 0), stop=False,
                            skip_group_check=True)
                if not with_f8:
                    return
                if is_score:
                    for kk in range(KP8):
                        lhsT8 = w8_sb[oc][:, kk, :, :]
                        for c in chunks:
                            nc.tensor.matmul(
                                pss[c], lhsT=lhsT8,
                                rhs=x8ts[kk][:, :,
                                             off + 512 * c:
                                             off + 512 * c + 512],
                                start=False, stop=(kk == KP8 - 1),
                                perf_mode=DR, skip_group_check=True)
                else:
                    for c in chunks:
                        nc.tensor.matmul(
                            pss[c], lhsT=w8_sb[oc][:],
                            rhs=x8ts[KVP][:, :,
                                          off + 512 * c: off + 512 * c + 512],
                            start=False, stop=True,
                            perf_mode=DR, skip_group_check=True)

            def mm_oc(oc):
                pss = [pmm.tile([128, 512], FP, tag="ps", name="ps")
                       for _ in range(4)]
                emit_mms(oc, pss)
                return pss

            def drain_score(jj, hi, pss, st):
                ap_idx = 4 + jj if hi else jj
                for c in range(4):
                    t = tp.tile([128, 512], FP, tag="tt", name="tt")
                    nc.vector.tensor_tensor(
                        t[:].rearrange("p (b s) -> p b s", s=4),
                        pss[c][:].rearrange("p (b s) -> p b s", s=4),
                        ape_sb[:, ap_idx, None, :].to_broadcast((128, 128, 4)),
                        ALU.add)
                    if (not hi) and c == 0:
                        # first block of the shard: -inf for the missing
                        # previous block (no-op on odd cores)
                        nc.vector.tensor_tensor(t[:, 0:4], t[:, 0:4],
                                                fix_sb[:], ALU.add)
                    e = ep.tile([128, 512], BF, tag="et", name="et")
                    nc.scalar.activation(e[:], t[:], ACTF.Exp,
                                         scale=1.0 / SCALE)
                    d = sp.tile([128, 128], FP, name="sden",
                                tag="sdh" if hi else "sden",
                                bufs=2 if hi else 5)
                    nc.vector.reduce_sum(
                        d[:], e[:].rearrange("p (b s) -> p b s", s=4),
                        axis=AX.X)
                    if hi:
                        nc.vector.tensor_tensor(st["den"][c][:],
                                                st["den"][c][:], d[:],
                                                ALU.add)
                        st["e_hi"][c] = e
                    else:
                        st["den"][c] = d
                        st["e_lo"][c] = e

            pending = []

            def drain_kv_chunk(jj, hi, c, ps_c, st):
                """Vector part of a kv drain; transpose+copy deferred."""
                e = st["e_hi" if hi else "e_lo"][c]
                p = pp.tile([128, 512], BF, tag="pt", name="pt")
                nc.vector.tensor_tensor(p[:], e[:], ps_c[:], ALU.mult)
                n = sp.tile([128, 128], FP, name="snum",
                            tag="snh" if hi else "snum",
                            bufs=2 if hi else 5)
                nc.vector.reduce_sum(
                    n[:], p[:].rearrange("p (b s) -> p b s", s=4), axis=AX.X)
                if hi:
                    nc.vector.tensor_tensor(st["num"][c][:], st["num"][c][:],
                                            n[:], ALU.add)
                    inv = sp.tile([128, 128], FP, tag="sinv", name="sinv",
                                  bufs=2)
                    nc.vector.reciprocal(inv[:], st["den"][c][:])
                    pooled = sp.tile([128, 128], FP, tag="spool",
                                     name="spool", bufs=6)
                    nc.vector.tensor_tensor(pooled[:], st["num"][c][:],
                                            inv[:], ALU.mult)
                    pending.append((jj, c, pooled))
                else:
                    st["num"][c] = n

            def flush_pending():
                # [channel, block] -> [block, channel]; runs when the
                # tensor engine reaches it, long after `pooled` is ready
                while pending:
                    jj, c, pooled = pending.pop(0)
                    trp = pmm.tile([128, 512], FP, tag="ps", name="trp")
                    nc.tensor.transpose(trp[:, 0:128], pooled[:], ident[:])
                    nc.scalar.copy(ys[c][:, 128 * jj: 128 * jj + 128],
                                   trp[:, 0:128])

            def epilogue(c):
                y = ys[c]
                # RMSNorm over the 512 channels
                sq = pp.tile([128, 512], BF, tag="pt", name="sq")
                nc.vector.tensor_tensor(sq[:], y[:], y[:], ALU.mult)
                ssum = sp.tile([128, 1], FP, tag="s1", name="ssum", bufs=3)
                nc.vector.reduce_sum(ssum[:], sq[:], axis=AX.X)
                rs = sp.tile([128, 1], FP, tag="s1", name="rs", bufs=3)
                nc.scalar.activation(rs[:], ssum[:], ACTF.Sqrt,
                                     bias=eps_sb[:], scale=1.0 / D)
                inv_rs = sp.tile([128, 1], FP, tag="s1", name="invrs", bufs=3)
                nc.vector.reciprocal(inv_rs[:], rs[:])
                nc.vector.tensor_scalar_mul(y[:], y[:], inv_rs[:])
                nc.vector.tensor_tensor(y[:], y[:], norm_sb[:], ALU.mult)
                # RoPE on the last 64 channels
                yr = y[:, 448:512].rearrange("p (m two) -> p m two", two=2)
                a, b = yr[:, :, 0], yr[:, :, 1]
                t1 = sp.tile([128, 32], FP, tag="s1", name="t1", bufs=3)
                t2 = sp.tile([128, 32], FP, tag="s1", name="t2", bufs=3)
                t3 = sp.tile([128, 32], FP, tag="s1", name="t3", bufs=3)
                t4 = sp.tile([128, 32], FP, tag="s1", name="t4", bufs=3)
                nc.vector.tensor_tensor(t1[:], a, cos_sb[c][:], ALU.mult)
                nc.vector.tensor_tensor(t2[:], b, sin_sb[c][:], ALU.mult)
                nc.vector.tensor_tensor(t3[:], a, sin_sb[c][:], ALU.mult)
                nc.vector.tensor_tensor(t4[:], b, cos_sb[c][:], ALU.mult)
                nc.vector.tensor_tensor(a, t1[:], t2[:], ALU.subtract)
                nc.vector.tensor_tensor(b, t3[:], t4[:], ALU.add)
                yo = tp.tile([128, 512], FP, tag="tt", name="yo")
                nc.scalar.copy(yo[:], y[:])
                nc.sync.dma_start(out[128 * c: 128 * c + 128, :], yo[:])

            st_by_fam = {jj: {"e_lo": {}, "e_hi": {}, "den": {}, "num": {}}
                         for jj in range(4)}

            # phase A: the two score ocs of family 0, k-interleaved so the
            # x stream stays ahead
            pss8 = [pmm.tile([128, 512], FP, tag="ps", name="ps")
                    for _ in range(4)]
            pss12 = [pmm.tile([128, 512], FP, tag="ps", name="ps")
                     for _ in range(4)]
            for k in range(KBF):
                emit_mms(8, pss8, ks=[k], with_f8=False)
                emit_mms(12, pss12, ks=[k], with_f8=False)
            for kk in range(KP8):
                for oc, pss in ((8, pss8), (12, pss12)):
                    lhsT8 = w8_sb[oc][:, kk, :, :]
                    off = _oc_off(oc)
                    for c in range(4):
                        nc.tensor.matmul(
                            pss[c], lhsT=lhsT8,
                            rhs=x8ts[kk][:, :,
                                         off + 512 * c: off + 512 * c + 512],
                            start=False, stop=(kk == KP8 - 1),
                            perf_mode=DR, skip_group_check=True)
            load_w(OC_ORDER[2], nc.sync)
            drain_score(0, False, pss8, st_by_fam[0])
            load_w(OC_ORDER[3], nc.scalar)
            drain_score(0, True, pss12, st_by_fam[0])

            # steady state: one oc at a time, 4+4 psum ping-pong
            for i in range(2, 16):
                oc = OC_ORDER[i]
                jj = oc % 4
                if i + 2 < 16:
                    load_w(OC_ORDER[i + 2], nc.sync if i % 2 == 0
                           else nc.scalar)
                st = st_by_fam[jj]
                if i == 15:
                    # last oc: chunk-major (pays ~160ns/matmul of exposed
                    # LdWeights, but every drain + epilogue except the very
                    # last overlaps the remaining matmul stream -- measured
                    # faster than k-major variants, whose bunched drain
                    # tail triggers the PE half-clock activity throttle)
                    for c in range(4):
                        ps_c = pmm.tile([128, 512], FP, tag="ps", name="ps")
                        emit_mms(oc, {c: ps_c}, chunks=(c,))
                        if c > 0:
                            flush_pending()
                            epilogue(c - 1)
                        drain_kv_chunk(jj, True, c, ps_c, st)
                    flush_pending()
                    epilogue(3)
                else:
                    pss = mm_oc(oc)
                    flush_pending()
                    if oc >= 12:
                        drain_score(jj, True, pss, st)
                    elif oc >= 8:
                        drain_score(jj, False, pss, st)
                    else:
                        for c in range(4):
                            drain_kv_chunk(jj, oc >= 4, c, pss[c], st)

    nc.finalize()
    return nc


_PROGRAM = None


def _get_program() -> bass.Bass:
    global _PROGRAM
    if _PROGRAM is None:
        _PROGRAM = _build_program()
    return _PROGRAM


def host_prep(inputs) -> list[dict]:
    x = np.asarray(inputs["x"], dtype=np.float32)
    wkv = np.asarray(inputs["wkv_w"], dtype=np.float32)
    wg = np.asarray(inputs["wgate_w"], dtype=np.float32)
    ape = np.asarray(inputs["ape"], dtype=np.float32)
    norm_w = np.asarray(inputs["norm_w"], dtype=np.float32)
    cos = np.asarray(inputs["cos"], dtype=np.float32)
    sin = np.asarray(inputs["sin"], dtype=np.float32)

    # bf16 weights all pre-scaled by SCALE (exact power of 2); the kv
    # path's scale is absorbed by the RMSNorm epilogue
    W_cat = np.concatenate([wkv, wg], axis=0) * SCALE  # [2048, 4096]
    # w_prep[oc, ki, kt, m] = W_cat[128*oc + m, 128*kt + ki]
    w_prep = np.ascontiguousarray(
        W_cat.reshape(16, 128, 32, 128).transpose(0, 3, 2, 1)
        .astype(ml_dtypes.bfloat16))
    # fp8 gate weights for K dims [128*KBF, 4096): [j, p, kk, t, m]
    wg8 = (wg[:, 128 * KBF:] * SW).astype(ml_dtypes.float8_e4m3fn)
    w8_prep = np.ascontiguousarray(
        wg8.reshape(8, 128, KP8, 2, 128).transpose(0, 4, 2, 3, 1))
    wk8 = (wkv[:, 128 * KVBF:] * SW).astype(ml_dtypes.float8_e4m3fn)
    w8k_prep = np.ascontiguousarray(
        wk8.reshape(8, 128, 2, 128).transpose(0, 3, 2, 1))
    ape_prep = np.ascontiguousarray(ape.T.reshape(8, 128, 4) * SCALE)
    cos_s = np.ascontiguousarray(cos[::RATIO][: S // RATIO])   # [1024, 32]
    sin_s = np.ascontiguousarray(sin[::RATIO][: S // RATIO])
    norm_b = np.ascontiguousarray(
        np.broadcast_to(norm_w[None, :], (128, 512))
        .astype(ml_dtypes.bfloat16))
    fix_neg = np.full((128, 4), NEG * SCALE, np.float32)
    fix_zero = np.zeros((128, 4), np.float32)

    in_maps = []
    for core in range(NCORES):
        b, half = core // 2, core % 2
        xb = x[b]
        if half == 0:
            xs = np.concatenate(
                [np.zeros((4, DIM), np.float32), xb[:TOK]], axis=0)
        else:
            xs = xb[TOK - 4: 2 * TOK]
        xT = np.ascontiguousarray(xs.T)                     # [4096, XW]
        xT_bf = np.ascontiguousarray(
            xT[:128 * KVBF].astype(ml_dtypes.bfloat16)
            .reshape(KVBF, 128, XW))
        # fp8 copy of the high-K dims: [kk, p, t, col]
        x8_prep = np.ascontiguousarray(
            (xT[128 * KBF:] * SX).astype(ml_dtypes.float8_e4m3fn)
            .reshape(KP8, 2, 128, XW).transpose(0, 2, 1, 3))
        in_maps.append(dict(
            xt=xT_bf,
            x8=x8_prep,
            w=w_prep,
            w8=w8_prep,
            w8k=w8k_prep,
            ape=ape_prep,
            cosp=np.ascontiguousarray(cos_s[half * 512: half * 512 + 512]),
            sinp=np.ascontiguousarray(sin_s[half * 512: half * 512 + 512]),
            normb=norm_b,
            scfix=(fix_neg if half == 0 else fix_zero),
        ))
    return in_maps


def assemble(results) -> np.ndarray:
    full = np.zeros((B, S // RATIO, D), np.float32)
    for core in range(NCORES):
        b, half = core // 2, core % 2
        full[b, half * 512: half * 512 + 512] = results[core]["out"]
    return full


def kernel(**inputs) -> np.ndarray:
    import os
    nc = _get_program()
    in_maps = host_prep(inputs)
    # force the plain execute path: a stray BASS_TRACE would route through
    # profiling hooks this environment may not have
    prev = os.environ.get("BASS_NEVER_TRACE")
    os.environ["BASS_NEVER_TRACE"] = "1"
    try:
        res = run_bass_kernel_spmd(nc, in_maps, list(range(NCORES)))
    finally:
        if prev is None:
            os.environ.pop("BASS_NEVER_TRACE", None)
        else:
            os.environ["BASS_NEVER_TRACE"] = prev
    return assemble(res.results)


# revision 12
# speedup vs baseline: 1.0611x; 1.0024x over previous
"""Trainium2 Bass kernel for the Compressor module (sparse-attention KV
compression): fused kv/score projections -> overlapped softmax pooling ->
RMSNorm -> RoPE.

Sharding: data-parallel over (batch x seq-half) across 8 cores. Each core
processes 2048 tokens of one batch with a 4-token halo at the front, so no
collectives are needed. Weights are replicated.

Layout: matmuls compute out.T = W @ x.T ([channel, token]). x.T is resident
in SBUF (32 bf16 k-rows of [128, 2052]); weight tiles stream through a
3-deep pool, each used for 4 moving matmuls (one per 512-token chunk), so
LdWeights is amortized 4x. One output-channel tile (4 psum banks) is in
flight at a time, ping-ponging with the previous tile's drain. The
"overlap" halves come from a 4-token shifted matmul window (lo channels
cols [0,2048), hi cols [4,2052)), which makes every softmax-pool group of
4 column-aligned.

Precision: kv matmuls all bf16 (~0.3% err). Score matmuls are split-K:
first 2560 dims bf16 with weights pre-scaled by 2048, last 1536 dims in
fp8-e4m3 DoubleRow perf mode (2 K-rows/cycle) with x*8 / w*256 scaling --
the psum ends up at 2048x scale, undone for free by the Exp activation's
scale argument. Pool/epilogue intermediates in bf16 where harmless.
Predicted rel err ~1.5e-2 vs the 2e-2 budget (validated numerically
against the real inputs; the numeric sim matches hardware to 6 digits).

Scheduling: PE-array transposes are deferred until after the NEXT oc's
matmuls are emitted so the tensor queue never waits on the vector drain
chain (which would drop the PE to half-clock pstate). The last oc runs
chunk-major with drains one chunk behind for the same reason. Weight DMAs
ride the fast sync/scalar queues ahead of the x stream; a tiny warm-up
matmul gated on x-row 3 delays the tensor start just enough that the x
stream stays ahead of consumption.
"""

import numpy as np
import ml_dtypes

import concourse.bass as bass
import concourse.mybir as mybir
from concourse import bacc
from concourse.tile import TileContext
from concourse.masks import make_identity
from concourse.bass_utils import run_bass_kernel_spmd

B, S, DIM = 4, 4096, 4096
D, RD, RATIO = 512, 64, 4
EPS = 1e-6
NCORES = 8
TOK = 2048          # tokens per core
XW = TOK + 4        # x window width (4-token halo at the front)
KBF = 20            # bf16 k-tiles for score ocs
KP8 = 6             # fp8 DoubleRow k-pair-tiles for score ocs
KVBF = 30           # bf16 k-tiles for kv ocs (tiles 30,31 in fp8)
KVP = (KVBF - KBF) // 2   # x8 pair index covering tiles 30,31
SCALE = 2048.0      # score psum scale (SX * SW)
SX, SW = 8.0, 256.0
NEG = -1.0e30
FP = mybir.dt.float32
BF = mybir.dt.bfloat16
F8 = mybir.dt.float8e4
DR = mybir.MatmulPerfMode.DoubleRow
AX = mybir.AxisListType
ALU = mybir.AluOpType
ACTF = mybir.ActivationFunctionType

# oc tiles: 0..7 = kv channels (lo 0..3, hi 4..7), 8..15 = gate/score
# channels (lo 8..11, hi 12..15). Family jj uses {jj, 4+jj, 8+jj, 12+jj}.
OC_ORDER = [8, 12, 0, 4,
            9, 13, 1, 5,
            10, 14, 2, 6,
            11, 15, 3, 7]


def _oc_off(oc: int) -> int:
    """Token-window offset: lo channels read cols [0,2048), hi [4,2052)."""
    return 4 if (oc // 4) % 2 == 1 else 0


def _build_program() -> bass.Bass:
    nc = bacc.Bacc("TRN2", target_bir_lowering=False, debug=False)

    xt = nc.dram_tensor("xt", [KVBF, 128, XW], BF,
                    kind="ExternalInput").ap()
    x8 = nc.dram_tensor("x8", [KP8, 128, 2, XW], F8,
                        kind="ExternalInput").ap()
    w = nc.dram_tensor("w", [16, 128, 32, 128], BF,
                       kind="ExternalInput").ap()
    w8 = nc.dram_tensor("w8", [8, 128, KP8, 2, 128], F8,
                        kind="ExternalInput").ap()
    w8k = nc.dram_tensor("w8k", [8, 128, 2, 128], F8,
                         kind="ExternalInput").ap()
    ape = nc.dram_tensor("ape", [8, 128, 4], FP, kind="ExternalInput").ap()
    cosp = nc.dram_tensor("cosp", [512, 32], FP, kind="ExternalInput").ap()
    sinp = nc.dram_tensor("sinp", [512, 32], FP, kind="ExternalInput").ap()
    normb = nc.dram_tensor("normb", [128, 512], BF, kind="ExternalInput").ap()
    scfix = nc.dram_tensor("scfix", [128, 4], FP, kind="ExternalInput").ap()
    out = nc.dram_tensor("out", [512, 512], FP, kind="ExternalOutput").ap()

    with TileContext(nc) as tc:
        with (
            tc.tile_pool(name="const", bufs=1) as constp,
            tc.tile_pool(name="xp", bufs=KVBF) as xp,
            tc.tile_pool(name="x8p", bufs=KP8) as x8p,
            tc.tile_pool(name="wp", bufs=3) as wp,
            tc.tile_pool(name="w8p", bufs=2) as w8p,
            tc.tile_pool(name="ep", bufs=8) as ep,
            tc.tile_pool(name="tp", bufs=2) as tp,
            tc.tile_pool(name="pp", bufs=2) as pp,
            tc.tile_pool(name="sp", bufs=6) as sp,
            tc.tile_pool(name="yp", bufs=4) as yp,
            tc.tile_pool(name="pmm", bufs=8, space="PSUM") as pmm,
        ):
            ident = constp.tile([128, 128], FP)
            make_identity(nc, ident)
            norm_sb = constp.tile([128, 512], BF)
            nc.gpsimd.dma_start(norm_sb, normb)
            fix_sb = constp.tile([128, 4], FP)
            nc.gpsimd.dma_start(fix_sb, scfix)
            ape_sb = constp.tile([128, 8, 4], FP)
            nc.gpsimd.dma_start(ape_sb, ape.rearrange("j p q -> p j q"))
            eps_sb = constp.tile([128, 1], FP)
            nc.gpsimd.memset(eps_sb[:], EPS)
            cos_sb, sin_sb = {}, {}
            for c in range(4):
                t = constp.tile([128, 32], FP, name=f"cos{c}")
                nc.gpsimd.dma_start(t, cosp[128 * c: 128 * c + 128, :])
                cos_sb[c] = t
                t = constp.tile([128, 32], FP, name=f"sin{c}")
                nc.gpsimd.dma_start(t, sinp[128 * c: 128 * c + 128, :])
                sin_sb[c] = t

            w_sb, w8_sb = {}, {}

            def load_w(oc, eng):
                t = wp.tile([128, KVBF, 128], BF, tag="wt", name="wt")
                if oc < 8:
                    eng.dma_start(t, w[oc, :, 0:KVBF, :])
                    t8 = w8p.tile([128, 2, 128], F8, tag="w8k",
                                  name="w8kt", bufs=2)
                    eng.dma_start(t8, w8k[oc])
                    w8_sb[oc] = t8
                else:
                    eng.dma_start(t[:, 0:KBF, :], w[oc, :, 0:KBF, :])
                    t8 = w8p.tile([128, KP8, 2, 128], F8, tag="w8",
                                  name="w8t")
                    eng.dma_start(t8, w8[oc - 8])
                    w8_sb[oc] = t8
                w_sb[oc] = t

            # weight tiles for the first two ocs at the head of the two
            # fast DMA queues, then the x stream (fp8 rows first: small,
            # and needed mid-phase-A)
            load_w(OC_ORDER[0], nc.sync)
            load_w(OC_ORDER[1], nc.scalar)

            x8ts = []
            for kk in range(KP8):
                t = x8p.tile([128, 2, XW], F8, tag="x8t", name="x8tile")
                eng = nc.sync if kk % 2 == 0 else nc.scalar
                eng.dma_start(t, x8[kk])
                x8ts.append(t)
            xts = []
            for k in range(KVBF):
                t = xp.tile([128, XW], BF, tag="xt", name="xtile")
                eng = nc.sync if k % 2 == 0 else nc.scalar
                eng.dma_start(t, xt[k])
                xts.append(t)

            # warm-up: tiny matmul gated on x-row 3 so the tensor engine
            # wakes only once the x stream has a head start
            warm = pmm.tile([128, 512], FP, tag="ps", name="warm")
            nc.tensor.matmul(warm[0:8, 0:8], lhsT=w_sb[8][:, 0, 0:8],
                             rhs=xts[3][:, 0:8], start=True, stop=True,
                             skip_group_check=True)

            ys = {}
            for c in range(4):
                ys[c] = yp.tile([128, 512], BF, tag="y", name="y")

            def emit_mms(oc, pss, ks=None, chunks=(0, 1, 2, 3),
                         with_f8=True):
                """Matmul stream for one oc into per-chunk psums."""
                off = _oc_off(oc)
                is_score = oc >= 8
                nk = KBF if is_score else KVBF
                for k in (range(nk) if ks is None else ks):
                    lhsT = w_sb[oc][:, k, :]
                    for c in chunks:
                        nc.tensor.matmul(
                            pss[c], lhsT=lhsT,
                            rhs=xts[k][:, off + 512 * c: off + 512 * c + 512],
                            start=(k == 0), stop=False,
                            skip_group_check=True)
                if not with_f8:
                    return
                if is_score:
                    for kk in range(KP8):
                        lhsT8 = w8_sb[oc][:, kk, :, :]
                        for c in chunks:
                            nc.tensor.matmul(
                                pss[c], lhsT=lhsT8,
                                rhs=x8ts[kk][:, :,
                                             off + 512 * c:
                                             off + 512 * c + 512],
                                start=False, stop=(kk == KP8 - 1),
                                perf_mode=DR, skip_group_check=True)
                else:
                    for c in chunks:
                        nc.tensor.matmul(
                            pss[c], lhsT=w8_sb[oc][:],
                            rhs=x8ts[KVP][:, :,
                                          off + 512 * c: off + 512 * c + 512],
                            start=False, stop=True,
                            perf_mode=DR, skip_group_check=True)

            def mm_oc(oc):
                pss = [pmm.tile([128, 512], FP, tag="ps", name="ps")
                       for _ in range(4)]
                emit_mms(oc, pss)
                return pss

            def drain_score(jj, hi, pss, st):
                ap_idx = 4 + jj if hi else jj
                for c in range(4):
                    t = tp.tile([128, 512], FP, tag="tt", name="tt")
                    nc.vector.tensor_tensor(
                        t[:].rearrange("p (b s) -> p b s", s=4),
                        pss[c][:].rearrange("p (b s) -> p b s", s=4),
                        ape_sb[:, ap_idx, None, :].to_broadcast((128, 128, 4)),
                        ALU.add)
                    if (not hi) and c == 0:
                        # first block of the shard: -inf for the missing
                        # previous block (no-op on odd cores)
                        nc.vector.tensor_tensor(t[:, 0:4], t[:, 0:4],
                                                fix_sb[:], ALU.add)
                    e = ep.tile([128, 512], BF, tag="et", name="et")
                    nc.scalar.activation(e[:], t[:], ACTF.Exp,
                                         scale=1.0 / SCALE)
                    d = sp.tile([128, 128], FP, name="sden",
                                tag="sdh" if hi else "sden",
                                bufs=2 if hi else 5)
                    nc.vector.reduce_sum(
                        d[:], e[:].rearrange("p (b s) -> p b s", s=4),
                        axis=AX.X)
                    if hi:
                        nc.vector.tensor_tensor(st["den"][c][:],
                                                st["den"][c][:], d[:],
                                                ALU.add)
                        st["e_hi"][c] = e
                    else:
                        st["den"][c] = d
                        st["e_lo"][c] = e

            pending = []

            def drain_kv_chunk(jj, hi, c, ps_c, st):
                """Vector part of a kv drain; transpose+copy deferred."""
                e = st["e_hi" if hi else "e_lo"][c]
                p = pp.tile([128, 512], BF, tag="pt", name="pt")
                nc.vector.tensor_tensor(p[:], e[:], ps_c[:], ALU.mult)
                n = sp.tile([128, 128], FP, name="snum",
                            tag="snh" if hi else "snum",
                            bufs=2 if hi else 5)
                nc.vector.reduce_sum(
                    n[:], p[:].rearrange("p (b s) -> p b s", s=4), axis=AX.X)
                if hi:
                    nc.vector.tensor_tensor(st["num"][c][:], st["num"][c][:],
                                            n[:], ALU.add)
                    inv = sp.tile([128, 128], FP, tag="sinv", name="sinv",
                                  bufs=2)
                    nc.vector.reciprocal(inv[:], st["den"][c][:])
                    pooled = sp.tile([128, 128], FP, tag="spool",
                                     name="spool", bufs=6)
                    nc.vector.tensor_tensor(pooled[:], st["num"][c][:],
                                            inv[:], ALU.mult)
                    pending.append((jj, c, pooled))
                else:
                    st["num"][c] = n

            def flush_pending():
                # [channel, block] -> [block, channel]; runs when the
                # tensor engine reaches it, long after `pooled` is ready
                while pending:
                    jj, c, pooled = pending.pop(0)
                    trp = pmm.tile([128, 512], FP, tag="ps", name="trp")
                    nc.tensor.transpose(trp[:, 0:128], pooled[:], ident[:])
                    nc.scalar.copy(ys[c][:, 128 * jj: 128 * jj + 128],
                                   trp[:, 0:128])

            def epilogue(c):
                y = ys[c]
                # RMSNorm over the 512 channels
                sq = pp.tile([128, 512], BF, tag="pt", name="sq")
                nc.vector.tensor_tensor(sq[:], y[:], y[:], ALU.mult)
                ssum = sp.tile([128, 1], FP, tag="s1", name="ssum", bufs=3)
                nc.vector.reduce_sum(ssum[:], sq[:], axis=AX.X)
                rs = sp.tile([128, 1], FP, tag="s1", name="rs", bufs=3)
                nc.scalar.activation(rs[:], ssum[:], ACTF.Sqrt,
                                     bias=eps_sb[:], scale=1.0 / D)
                inv_rs = sp.tile([128, 1], FP, tag="s1", name="invrs", bufs=3)
                nc.vector.reciprocal(inv_rs[:], rs[:])
                nc.vector.tensor_scalar_mul(y[:], y[:], inv_rs[:])
                nc.vector.tensor_tensor(y[:], y[:], norm_sb[:], ALU.mult)
                # RoPE on the last 64 channels
                yr = y[:, 448:512].rearrange("p (m two) -> p m two", two=2)
                a, b = yr[:, :, 0], yr[:, :, 1]
                t1 = sp.tile([128, 32], FP, tag="s1", name="t1", bufs=3)
                t2 = sp.tile([128, 32], FP, tag="s1", name="t2", bufs=3)
                t3 = sp.tile([128, 32], FP, tag="s1", name="t3", bufs=3)
                t4 = sp.tile([128, 32], FP, tag="s1", name="t4", bufs=3)
                nc.vector.tensor_tensor(t1[:], a, cos_sb[c][:], ALU.mult)
                nc.vector.tensor_tensor(t2[:], b, sin_sb[c][:], ALU.mult)
                nc.vector.tensor_tensor(t3[:], a, sin_sb[c][:], ALU.mult)
                nc.vector.tensor_tensor(t4[:], b, cos_sb[c][:], ALU.mult)
                nc.vector.tensor_tensor(a, t1[:], t2[:], ALU.subtract)
                nc.vector.tensor_tensor(b, t3[:], t4[:], ALU.add)
                yo = tp.tile([128, 512], FP, tag="tt", name="yo")
                nc.scalar.copy(yo[:], y[:])
                nc.sync.dma_start(out[128 * c: 128 * c + 128, :], yo[:])

            st_by_fam = {jj: {"e_lo": {}, "e_hi": {}, "den": {}, "num": {}}
                         for jj in range(4)}

            # phase A: the two score ocs of family 0, k-interleaved so the
            # x stream stays ahead
            pss8 = [pmm.tile([128, 512], FP, tag="ps", name="ps")
                    for _ in range(4)]
            pss12 = [pmm.tile([128, 512], FP, tag="ps", name="ps")
                     for _ in range(4)]
            for k in range(KBF):
                emit_mms(8, pss8, ks=[k], with_f8=False)
                emit_mms(12, pss12, ks=[k], with_f8=False)
            for kk in range(KP8):
                for oc, pss in ((8, pss8), (12, pss12)):
                    lhsT8 = w8_sb[oc][:, kk, :, :]
                    off = _oc_off(oc)
                    for c in range(4):
                        nc.tensor.matmul(
                            pss[c], lhsT=lhsT8,
                            rhs=x8ts[kk][:, :,
                                         off + 512 * c: off + 512 * c + 512],
                            start=False, stop=(kk == KP8 - 1),
                            perf_mode=DR, skip_group_check=True)
            load_w(OC_ORDER[2], nc.sync)
            drain_score(0, False, pss8, st_by_fam[0])
            load_w(OC_ORDER[3], nc.scalar)
            drain_score(0, True, pss12, st_by_fam[0])

            # steady state: one oc at a time, 4+4 psum ping-pong
            for i in range(2, 16):
                oc = OC_ORDER[i]
                jj = oc % 4
                if i + 2 < 16:
                    load_w(OC_ORDER[i + 2], nc.sync if i % 2 == 0
                           else nc.scalar)
                st = st_by_fam[jj]
                if i == 15:
                    # last oc: chunk-major (pays ~160ns/matmul of exposed
                    # LdWeights, but every drain + epilogue except the very
                    # last overlaps the remaining matmul stream -- measured
                    # faster than k-major variants, whose bunched drain
                    # tail triggers the PE half-clock activity throttle)
                    for c in range(4):
                        ps_c = pmm.tile([128, 512], FP, tag="ps", name="ps")
                        emit_mms(oc, {c: ps_c}, chunks=(c,))
                        if c > 0:
                            flush_pending()
                            epilogue(c - 1)
                        drain_kv_chunk(jj, True, c, ps_c, st)
                    flush_pending()
                    epilogue(3)
                else:
                    pss = mm_oc(oc)
                    flush_pending()
                    if oc >= 12:
                        drain_score(jj, True, pss, st)
                    elif oc >= 8:
                        drain_score(jj, False, pss, st)
                    else:
                        for c in range(4):
                            drain_kv_chunk(jj, oc >= 4, c, pss[c], st)

    nc.finalize()
    return nc


_PROGRAM = None


def _get_program() -> bass.Bass:
    global _PROGRAM
    if _PROGRAM is None:
        _PROGRAM = _build_program()
    return _PROGRAM


def host_prep(inputs) -> list[dict]:
    x = np.asarray(inputs["x"], dtype=np.float32)
    wkv = np.asarray(inputs["wkv_w"], dtype=np.float32)
    wg = np.asarray(inputs["wgate_w"], dtype=np.float32)
    ape = np.asarray(inputs["ape"], dtype=np.float32)
    norm_w = np.asarray(inputs["norm_w"], dtype=np.float32)
    cos = np.asarray(inputs["cos"], dtype=np.float32)
    sin = np.asarray(inputs["sin"], dtype=np.float32)

    # bf16 weights all pre-scaled by SCALE (exact power of 2); the kv
    # path's scale is absorbed by the RMSNorm epilogue
    W_cat = np.concatenate([wkv, wg], axis=0) * SCALE  # [2048, 4096]
    # w_prep[oc, ki, kt, m] = W_cat[128*oc + m, 128*kt + ki]
    w_prep = np.ascontiguousarray(
        W_cat.reshape(16, 128, 32, 128).transpose(0, 3, 2, 1)
        .astype(ml_dtypes.bfloat16))
    # fp8 gate weights for K dims [128*KBF, 4096): [j, p, kk, t, m]
    wg8 = (wg[:, 128 * KBF:] * SW).astype(ml_dtypes.float8_e4m3fn)
    w8_prep = np.ascontiguousarray(
        wg8.reshape(8, 128, KP8, 2, 128).transpose(0, 4, 2, 3, 1))
    wk8 = (wkv[:, 128 * KVBF:] * SW).astype(ml_dtypes.float8_e4m3fn)
    w8k_prep = np.ascontiguousarray(
        wk8.reshape(8, 128, 2, 128).transpose(0, 3, 2, 1))
    ape_prep = np.ascontiguousarray(ape.T.reshape(8, 128, 4) * SCALE)
    cos_s = np.ascontiguousarray(cos[::RATIO][: S // RATIO])   # [1024, 32]
    sin_s = np.ascontiguousarray(sin[::RATIO][: S // RATIO])
    norm_b = np.ascontiguousarray(
        np.broadcast_to(norm_w[None, :], (128, 512))
        .astype(ml_dtypes.bfloat16))
    fix_neg = np.full((128, 4), NEG * SCALE, np.float32)
    fix_zero = np.zeros((128, 4), np.float32)

    in_maps = []
    for core in range(NCORES):
        b, half = core // 2, core % 2
        xb = x[b]
        if half == 0:
            xs = np.concatenate(
                [np.zeros((4, DIM), np.float32), xb[:TOK]], axis=0)
        else:
            xs = xb[TOK - 4: 2 * TOK]
        xT = np.ascontiguousarray(xs.T)                     # [4096, XW]
        xT_bf = np.ascontiguousarray(
            xT[:128 * KVBF].astype(ml_dtypes.bfloat16)
            .reshape(KVBF, 128, XW))
        # fp8 copy of the high-K dims: [kk, p, t, col]
        x8_prep = np.ascontiguousarray(
            (xT[128 * KBF:] * SX).astype(ml_dtypes.float8_e4m3fn)
            .reshape(KP8, 2, 128, XW).transpose(0, 2, 1, 3))
        in_maps.append(dict(
            xt=xT_bf,
            x8=x8_prep,
            w=w_prep,
            w8=w8_prep,
            w8k=w8k_prep,
            ape=ape_prep,
            cosp=np.ascontiguousarray(cos_s[half * 512: half * 512 + 512]),
            sinp=np.ascontiguousarray(sin_s[half * 512: half * 512 + 512]),
            normb=norm_b,
            scfix=(fix_neg if half == 0 else fix_zero),
        ))
    return in_maps


def assemble(results) -> np.ndarray:
    full = np.zeros((B, S // RATIO, D), np.float32)
    for core in range(NCORES):
        b, half = core // 2, core % 2
        full[b, half * 512: half * 512 + 512] = results[core]["out"]
    return full


def kernel(**inputs) -> np.ndarray:
    import os
    nc = _get_program()
    in_maps = host_prep(inputs)
    # force the plain execute path: a stray BASS_TRACE would route through
    # profiling hooks this environment may not have
    prev = os.environ.get("BASS_NEVER_TRACE")
    os.environ["BASS_NEVER_TRACE"] = "1"
    try:
        res = run_bass_kernel_spmd(nc, in_maps, list(range(NCORES)))
    finally:
        if prev is None:
            os.environ.pop("BASS_NEVER_TRACE", None)
        else:
            os.environ["BASS_NEVER_TRACE"] = prev
    return assemble(res.results)


# revision 13
# speedup vs baseline: 1.0874x; 1.0248x over previous
"""Trainium2 Bass kernel for the Compressor module (sparse-attention KV
compression): fused kv/score projections -> overlapped softmax pooling ->
RMSNorm -> RoPE.

Sharding: data-parallel over (batch x seq-half) across 8 cores. Each core
processes 2048 tokens of one batch with a 4-token halo at the front, so no
collectives are needed. Weights are replicated.

Layout: matmuls compute out.T = W @ x.T ([channel, token]). x.T is resident
in SBUF (32 bf16 k-rows of [128, 2052]); weight tiles stream through a
3-deep pool, each used for 4 moving matmuls (one per 512-token chunk), so
LdWeights is amortized 4x. One output-channel tile (4 psum banks) is in
flight at a time, ping-ponging with the previous tile's drain. The
"overlap" halves come from a 4-token shifted matmul window (lo channels
cols [0,2048), hi cols [4,2052)), which makes every softmax-pool group of
4 column-aligned.

Precision: kv matmuls all bf16 (~0.3% err). Score matmuls are split-K:
first 2560 dims bf16 with weights pre-scaled by 2048, last 1536 dims in
fp8-e4m3 DoubleRow perf mode (2 K-rows/cycle) with x*8 / w*256 scaling --
the psum ends up at 2048x scale, undone for free by the Exp activation's
scale argument. Pool/epilogue intermediates in bf16 where harmless.
Predicted rel err ~1.5e-2 vs the 2e-2 budget (validated numerically
against the real inputs; the numeric sim matches hardware to 6 digits).

Scheduling: PE-array transposes are deferred until after the NEXT oc's
matmuls are emitted so the tensor queue never waits on the vector drain
chain (which would drop the PE to half-clock pstate). The last oc runs
chunk-major with drains one chunk behind for the same reason. Weight DMAs
ride the fast sync/scalar queues ahead of the x stream; a tiny warm-up
matmul gated on x-row 3 delays the tensor start just enough that the x
stream stays ahead of consumption.
"""

import numpy as np
import ml_dtypes

import concourse.bass as bass
import concourse.mybir as mybir
from concourse import bacc
from concourse.tile import TileContext
from concourse.masks import make_identity
from concourse.bass_utils import run_bass_kernel_spmd

B, S, DIM = 4, 4096, 4096
D, RD, RATIO = 512, 64, 4
EPS = 1e-6
NCORES = 8
TOK = 2048          # tokens per core
XW = TOK + 4        # x window width (4-token halo at the front)
KBF = 20            # bf16 k-tiles for score ocs
KP8 = 6             # fp8 DoubleRow k-pair-tiles for score ocs
KVBF = 30           # bf16 k-tiles for kv ocs (tiles 30,31 in fp8)
KVP = (KVBF - KBF) // 2   # x8 pair index covering tiles 30,31
SCALE = 2048.0      # score psum scale (SX * SW)
SX, SW = 8.0, 256.0
NEG = -1.0e30
FP = mybir.dt.float32
BF = mybir.dt.bfloat16
F8 = mybir.dt.float8e4
DR = mybir.MatmulPerfMode.DoubleRow
AX = mybir.AxisListType
ALU = mybir.AluOpType
ACTF = mybir.ActivationFunctionType

# oc tiles: 0..7 = kv channels (lo 0..3, hi 4..7), 8..15 = gate/score
# channels (lo 8..11, hi 12..15). Family jj uses {jj, 4+jj, 8+jj, 12+jj}.
OC_ORDER = [8, 12, 0, 4,
            9, 13, 1, 5,
            10, 14, 2, 6,
            11, 15, 3, 7]


def _oc_off(oc: int) -> int:
    """Token-window offset: lo channels read cols [0,2048), hi [4,2052)."""
    return 4 if (oc // 4) % 2 == 1 else 0


def _build_program() -> bass.Bass:
    nc = bacc.Bacc("TRN2", target_bir_lowering=False, debug=False)

    xt = nc.dram_tensor("xt", [KVBF, 128, XW], BF,
                    kind="ExternalInput").ap()
    x8 = nc.dram_tensor("x8", [KP8, 128, 2, XW], F8,
                        kind="ExternalInput").ap()
    w = nc.dram_tensor("w", [16, 128, 32, 128], BF,
                       kind="ExternalInput").ap()
    w8 = nc.dram_tensor("w8", [8, 128, KP8, 2, 128], F8,
                        kind="ExternalInput").ap()
    w8k = nc.dram_tensor("w8k", [8, 128, 2, 128], F8,
                         kind="ExternalInput").ap()
    ape = nc.dram_tensor("ape", [8, 128, 4], FP, kind="ExternalInput").ap()
    cosp = nc.dram_tensor("cosp", [512, 32], FP, kind="ExternalInput").ap()
    sinp = nc.dram_tensor("sinp", [512, 32], FP, kind="ExternalInput").ap()
    normb = nc.dram_tensor("normb", [128, 512], BF, kind="ExternalInput").ap()
    scfix = nc.dram_tensor("scfix", [128, 4], FP, kind="ExternalInput").ap()
    out = nc.dram_tensor("out", [512, 512], FP, kind="ExternalOutput").ap()

    with TileContext(nc) as tc:
        with (
            tc.tile_pool(name="const", bufs=1) as constp,
            tc.tile_pool(name="xp", bufs=KVBF) as xp,
            tc.tile_pool(name="x8p", bufs=KP8) as x8p,
            tc.tile_pool(name="wp", bufs=3) as wp,
            tc.tile_pool(name="w8p", bufs=2) as w8p,
            tc.tile_pool(name="ep", bufs=8) as ep,
            tc.tile_pool(name="tp", bufs=2) as tp,
            tc.tile_pool(name="pp", bufs=2) as pp,
            tc.tile_pool(name="sp", bufs=6) as sp,
            tc.tile_pool(name="yp", bufs=4) as yp,
            tc.tile_pool(name="pmm", bufs=8, space="PSUM") as pmm,
        ):
            ident = constp.tile([128, 128], FP)
            make_identity(nc, ident)
            norm_sb = constp.tile([128, 512], BF)
            nc.gpsimd.dma_start(norm_sb, normb)
            fix_sb = constp.tile([128, 4], FP)
            nc.gpsimd.dma_start(fix_sb, scfix)
            ape_sb = constp.tile([128, 8, 4], FP)
            nc.gpsimd.dma_start(ape_sb, ape.rearrange("j p q -> p j q"))
            eps_sb = constp.tile([128, 1], FP)
            nc.gpsimd.memset(eps_sb[:], EPS)
            cos_sb, sin_sb = {}, {}
            for c in range(4):
                t = constp.tile([128, 32], FP, name=f"cos{c}")
                nc.gpsimd.dma_start(t, cosp[128 * c: 128 * c + 128, :])
                cos_sb[c] = t
                t = constp.tile([128, 32], FP, name=f"sin{c}")
                nc.gpsimd.dma_start(t, sinp[128 * c: 128 * c + 128, :])
                sin_sb[c] = t

            w_sb, w8_sb = {}, {}

            def load_w(oc, eng):
                t = wp.tile([128, KVBF, 128], BF, tag="wt", name="wt")
                if oc < 8:
                    eng.dma_start(t, w[oc, :, 0:KVBF, :])
                    t8 = w8p.tile([128, 2, 128], F8, tag="w8k",
                                  name="w8kt", bufs=2)
                    eng.dma_start(t8, w8k[oc])
                    w8_sb[oc] = t8
                else:
                    eng.dma_start(t[:, 0:KBF, :], w[oc, :, 0:KBF, :])
                    t8 = w8p.tile([128, KP8, 2, 128], F8, tag="w8",
                                  name="w8t")
                    eng.dma_start(t8, w8[oc - 8])
                    w8_sb[oc] = t8
                w_sb[oc] = t

            # weight tiles for the first two ocs at the head of the two
            # fast DMA queues, then the x stream (fp8 rows first: small,
            # and needed mid-phase-A)
            load_w(OC_ORDER[0], nc.sync)
            load_w(OC_ORDER[1], nc.scalar)

            # bf16 rows 0..7 first (they gate the phase-A start), then the
            # fp8 rows (not consumed until the fp8 section ~50us in), then
            # the rest of the bf16 stream
            xts = [None] * KVBF
            x8ts = [None] * KP8

            def load_x(k):
                t = xp.tile([128, XW], BF, tag="xt", name="xtile")
                eng = nc.sync if k % 2 == 0 else nc.scalar
                eng.dma_start(t, xt[k])
                xts[k] = t

            for k in range(8):
                load_x(k)
            for kk in range(KP8):
                t = x8p.tile([128, 2, XW], F8, tag="x8t", name="x8tile")
                eng = nc.sync if kk % 2 == 0 else nc.scalar
                eng.dma_start(t, x8[kk])
                x8ts[kk] = t
            for k in range(8, KVBF):
                load_x(k)

            # warm-up: tiny matmul gated on x-row 3 so the tensor engine
            # wakes only once the x stream has a head start
            warm = pmm.tile([128, 512], FP, tag="ps", name="warm")
            nc.tensor.matmul(warm[0:8, 0:8], lhsT=w_sb[8][:, 0, 0:8],
                             rhs=xts[3][:, 0:8], start=True, stop=True,
                             skip_group_check=True)

            ys = {}
            for c in range(4):
                ys[c] = yp.tile([128, 512], BF, tag="y", name="y")

            def emit_mms(oc, pss, ks=None, chunks=(0, 1, 2, 3),
                         with_f8=True):
                """Matmul stream for one oc into per-chunk psums."""
                off = _oc_off(oc)
                is_score = oc >= 8
                nk = KBF if is_score else KVBF
                for k in (range(nk) if ks is None else ks):
                    lhsT = w_sb[oc][:, k, :]
                    for c in chunks:
                        nc.tensor.matmul(
                            pss[c], lhsT=lhsT,
                            rhs=xts[k][:, off + 512 * c: off + 512 * c + 512],
                            start=(k == 0), stop=False,
                            skip_group_check=True)
                if not with_f8:
                    return
                if is_score:
                    for kk in range(KP8):
                        lhsT8 = w8_sb[oc][:, kk, :, :]
                        for c in chunks:
                            nc.tensor.matmul(
                                pss[c], lhsT=lhsT8,
                                rhs=x8ts[kk][:, :,
                                             off + 512 * c:
                                             off + 512 * c + 512],
                                start=False, stop=(kk == KP8 - 1),
                                perf_mode=DR, skip_group_check=True)
                else:
                    for c in chunks:
                        nc.tensor.matmul(
                            pss[c], lhsT=w8_sb[oc][:],
                            rhs=x8ts[KVP][:, :,
                                          off + 512 * c: off + 512 * c + 512],
                            start=False, stop=True,
                            perf_mode=DR, skip_group_check=True)

            def mm_oc(oc):
                pss = [pmm.tile([128, 512], FP, tag="ps", name="ps")
                       for _ in range(4)]
                emit_mms(oc, pss)
                return pss

            def drain_score(jj, hi, pss, st):
                ap_idx = 4 + jj if hi else jj
                for c in range(4):
                    t = tp.tile([128, 512], FP, tag="tt", name="tt")
                    nc.vector.tensor_tensor(
                        t[:].rearrange("p (b s) -> p b s", s=4),
                        pss[c][:].rearrange("p (b s) -> p b s", s=4),
                        ape_sb[:, ap_idx, None, :].to_broadcast((128, 128, 4)),
                        ALU.add)
                    if (not hi) and c == 0:
                        # first block of the shard: -inf for the missing
                        # previous block (no-op on odd cores)
                        nc.vector.tensor_tensor(t[:, 0:4], t[:, 0:4],
                                                fix_sb[:], ALU.add)
                    e = ep.tile([128, 512], BF, tag="et", name="et")
                    nc.scalar.activation(e[:], t[:], ACTF.Exp,
                                         scale=1.0 / SCALE)
                    d = sp.tile([128, 128], FP, name="sden",
                                tag="sdh" if hi else "sden",
                                bufs=2 if hi else 5)
                    nc.vector.reduce_sum(
                        d[:], e[:].rearrange("p (b s) -> p b s", s=4),
                        axis=AX.X)
                    if hi:
                        nc.vector.tensor_tensor(st["den"][c][:],
                                                st["den"][c][:], d[:],
                                                ALU.add)
                        st["e_hi"][c] = e
                    else:
                        st["den"][c] = d
                        st["e_lo"][c] = e

            pending = []

            def drain_kv_chunk(jj, hi, c, ps_c, st):
                """Vector part of a kv drain; transpose+copy deferred."""
                e = st["e_hi" if hi else "e_lo"][c]
                p = pp.tile([128, 512], BF, tag="pt", name="pt")
                nc.vector.tensor_tensor(p[:], e[:], ps_c[:], ALU.mult)
                n = sp.tile([128, 128], FP, name="snum",
                            tag="snh" if hi else "snum",
                            bufs=2 if hi else 5)
                nc.vector.reduce_sum(
                    n[:], p[:].rearrange("p (b s) -> p b s", s=4), axis=AX.X)
                if hi:
                    nc.vector.tensor_tensor(st["num"][c][:], st["num"][c][:],
                                            n[:], ALU.add)
                    inv = sp.tile([128, 128], FP, tag="sinv", name="sinv",
                                  bufs=2)
                    nc.vector.reciprocal(inv[:], st["den"][c][:])
                    pooled = sp.tile([128, 128], FP, tag="spool",
                                     name="spool", bufs=6)
                    nc.vector.tensor_tensor(pooled[:], st["num"][c][:],
                                            inv[:], ALU.mult)
                    pending.append((jj, c, pooled))
                else:
                    st["num"][c] = n

            def flush_pending():
                # [channel, block] -> [block, channel]; runs when the
                # tensor engine reaches it, long after `pooled` is ready
                while pending:
                    jj, c, pooled = pending.pop(0)
                    trp = pmm.tile([128, 512], FP, tag="ps", name="trp")
                    nc.tensor.transpose(trp[:, 0:128], pooled[:], ident[:])
                    nc.scalar.copy(ys[c][:, 128 * jj: 128 * jj + 128],
                                   trp[:, 0:128])

            def epilogue(c):
                y = ys[c]
                # RMSNorm over the 512 channels
                sq = pp.tile([128, 512], BF, tag="pt", name="sq")
                nc.vector.tensor_tensor(sq[:], y[:], y[:], ALU.mult)
                ssum = sp.tile([128, 1], FP, tag="s1", name="ssum", bufs=3)
                nc.vector.reduce_sum(ssum[:], sq[:], axis=AX.X)
                rs = sp.tile([128, 1], FP, tag="s1", name="rs", bufs=3)
                nc.scalar.activation(rs[:], ssum[:], ACTF.Sqrt,
                                     bias=eps_sb[:], scale=1.0 / D)
                inv_rs = sp.tile([128, 1], FP, tag="s1", name="invrs", bufs=3)
                nc.vector.reciprocal(inv_rs[:], rs[:])
                nc.vector.tensor_scalar_mul(y[:], y[:], inv_rs[:])
                nc.vector.tensor_tensor(y[:], y[:], norm_sb[:], ALU.mult)
                # RoPE on the last 64 channels
                yr = y[:, 448:512].rearrange("p (m two) -> p m two", two=2)
                a, b = yr[:, :, 0], yr[:, :, 1]
                t1 = sp.tile([128, 32], FP, tag="s1", name="t1", bufs=3)
                t2 = sp.tile([128, 32], FP, tag="s1", name="t2", bufs=3)
                t3 = sp.tile([128, 32], FP, tag="s1", name="t3", bufs=3)
                t4 = sp.tile([128, 32], FP, tag="s1", name="t4", bufs=3)
                nc.vector.tensor_tensor(t1[:], a, cos_sb[c][:], ALU.mult)
                nc.vector.tensor_tensor(t2[:], b, sin_sb[c][:], ALU.mult)
                nc.vector.tensor_tensor(t3[:], a, sin_sb[c][:], ALU.mult)
                nc.vector.tensor_tensor(t4[:], b, cos_sb[c][:], ALU.mult)
                nc.vector.tensor_tensor(a, t1[:], t2[:], ALU.subtract)
                nc.vector.tensor_tensor(b, t3[:], t4[:], ALU.add)
                yo = tp.tile([128, 512], FP, tag="tt", name="yo")
                nc.scalar.copy(yo[:], y[:])
                nc.sync.dma_start(out[128 * c: 128 * c + 128, :], yo[:])

            st_by_fam = {jj: {"e_lo": {}, "e_hi": {}, "den": {}, "num": {}}
                         for jj in range(4)}

            # phase A: the two score ocs of family 0, k-interleaved so the
            # x stream stays ahead
            pss8 = [pmm.tile([128, 512], FP, tag="ps", name="ps")
                    for _ in range(4)]
            pss12 = [pmm.tile([128, 512], FP, tag="ps", name="ps")
                     for _ in range(4)]
            for k in range(KBF):
                emit_mms(8, pss8, ks=[k], with_f8=False)
                emit_mms(12, pss12, ks=[k], with_f8=False)
            for kk in range(KP8):
                for oc, pss in ((8, pss8), (12, pss12)):
                    lhsT8 = w8_sb[oc][:, kk, :, :]
                    off = _oc_off(oc)
                    for c in range(4):
                        nc.tensor.matmul(
                            pss[c], lhsT=lhsT8,
                            rhs=x8ts[kk][:, :,
                                         off + 512 * c: off + 512 * c + 512],
                            start=False, stop=(kk == KP8 - 1),
                            perf_mode=DR, skip_group_check=True)
            load_w(OC_ORDER[2], nc.sync)
            drain_score(0, False, pss8, st_by_fam[0])
            load_w(OC_ORDER[3], nc.scalar)
            drain_score(0, True, pss12, st_by_fam[0])

            # steady state: one oc at a time, 4+4 psum ping-pong
            for i in range(2, 16):
                oc = OC_ORDER[i]
                jj = oc % 4
                if i + 2 < 16:
                    load_w(OC_ORDER[i + 2], nc.sync if i % 2 == 0
                           else nc.scalar)
                st = st_by_fam[jj]
                if i == 15:
                    # last oc: chunk-major (pays ~160ns/matmul of exposed
                    # LdWeights, but every drain + epilogue except the very
                    # last overlaps the remaining matmul stream -- measured
                    # faster than k-major variants, whose bunched drain
                    # tail triggers the PE half-clock activity throttle)
                    for c in range(4):
                        ps_c = pmm.tile([128, 512], FP, tag="ps", name="ps")
                        emit_mms(oc, {c: ps_c}, chunks=(c,))
                        if c > 0:
                            flush_pending()
                            epilogue(c - 1)
                        drain_kv_chunk(jj, True, c, ps_c, st)
                    flush_pending()
                    epilogue(3)
                else:
                    pss = mm_oc(oc)
                    flush_pending()
                    if oc >= 12:
                        drain_score(jj, True, pss, st)
                    elif oc >= 8:
                        drain_score(jj, False, pss, st)
                    else:
                        for c in range(4):
                            drain_kv_chunk(jj, oc >= 4, c, pss[c], st)

    nc.finalize()
    return nc


_PROGRAM = None


def _get_program() -> bass.Bass:
    global _PROGRAM
    if _PROGRAM is None:
        _PROGRAM = _build_program()
    return _PROGRAM


def host_prep(inputs) -> list[dict]:
    x = np.asarray(inputs["x"], dtype=np.float32)
    wkv = np.asarray(inputs["wkv_w"], dtype=np.float32)
    wg = np.asarray(inputs["wgate_w"], dtype=np.float32)
    ape = np.asarray(inputs["ape"], dtype=np.float32)
    norm_w = np.asarray(inputs["norm_w"], dtype=np.float32)
    cos = np.asarray(inputs["cos"], dtype=np.float32)
    sin = np.asarray(inputs["sin"], dtype=np.float32)

    # bf16 weights all pre-scaled by SCALE (exact power of 2); the kv
    # path's scale is absorbed by the RMSNorm epilogue
    W_cat = np.concatenate([wkv, wg], axis=0) * SCALE  # [2048, 4096]
    # w_prep[oc, ki, kt, m] = W_cat[128*oc + m, 128*kt + ki]
    w_prep = np.ascontiguousarray(
        W_cat.reshape(16, 128, 32, 128).transpose(0, 3, 2, 1)
        .astype(ml_dtypes.bfloat16))
    # fp8 gate weights for K dims [128*KBF, 4096): [j, p, kk, t, m]
    wg8 = (wg[:, 128 * KBF:] * SW).astype(ml_dtypes.float8_e4m3fn)
    w8_prep = np.ascontiguousarray(
        wg8.reshape(8, 128, KP8, 2, 128).transpose(0, 4, 2, 3, 1))
    wk8 = (wkv[:, 128 * KVBF:] * SW).astype(ml_dtypes.float8_e4m3fn)
    w8k_prep = np.ascontiguousarray(
        wk8.reshape(8, 128, 2, 128).transpose(0, 3, 2, 1))
    ape_prep = np.ascontiguousarray(ape.T.reshape(8, 128, 4) * SCALE)
    cos_s = np.ascontiguousarray(cos[::RATIO][: S // RATIO])   # [1024, 32]
    sin_s = np.ascontiguousarray(sin[::RATIO][: S // RATIO])
    norm_b = np.ascontiguousarray(
        np.broadcast_to(norm_w[None, :], (128, 512))
        .astype(ml_dtypes.bfloat16))
    fix_neg = np.full((128, 4), NEG * SCALE, np.float32)
    fix_zero = np.zeros((128, 4), np.float32)

    in_maps = []
    for core in range(NCORES):
        b, half = core // 2, core % 2
        xb = x[b]
        if half == 0:
            xs = np.concatenate(
                [np.zeros((4, DIM), np.float32), xb[:TOK]], axis=0)
        else:
            xs = xb[TOK - 4: 2 * TOK]
        xT = np.ascontiguousarray(xs.T)                     # [4096, XW]
        xT_bf = np.ascontiguousarray(
            xT[:128 * KVBF].astype(ml_dtypes.bfloat16)
            .reshape(KVBF, 128, XW))
        # fp8 copy of the high-K dims: [kk, p, t, col]
        x8_prep = np.ascontiguousarray(
            (xT[128 * KBF:] * SX).astype(ml_dtypes.float8_e4m3fn)
            .reshape(KP8, 2, 128, XW).transpose(0, 2, 1, 3))
        in_maps.append(dict(
            xt=xT_bf,
            x8=x8_prep,
            w=w_prep,
            w8=w8_prep,
            w8k=w8k_prep,
            ape=ape_prep,
            cosp=np.ascontiguousarray(cos_s[half * 512: half * 512 + 512]),
            sinp=np.ascontiguousarray(sin_s[half * 512: half * 512 + 512]),
            normb=norm_b,
            scfix=(fix_neg if half == 0 else fix_zero),
        ))
    return in_maps


def assemble(results) -> np.ndarray:
    full = np.zeros((B, S // RATIO, D), np.float32)
    for core in range(NCORES):
        b, half = core // 2, core % 2
        full[b, half * 512: half * 512 + 512] = results[core]["out"]
    return full


def kernel(**inputs) -> np.ndarray:
    import os
    nc = _get_program()
    in_maps = host_prep(inputs)
    # force the plain execute path: a stray BASS_TRACE would route through
    # profiling hooks this environment may not have
    prev = os.environ.get("BASS_NEVER_TRACE")
    os.environ["BASS_NEVER_TRACE"] = "1"
    try:
        res = run_bass_kernel_spmd(nc, in_maps, list(range(NCORES)))
    finally:
        if prev is None:
            os.environ.pop("BASS_NEVER_TRACE", None)
        else:
            os.environ["BASS_NEVER_TRACE"] = prev
    return assemble(res.results)
